# revision 46
# baseline (speedup 1.0000x reference)
"""Self-contained Trainium2 Bass kernel for nn_Att_MambaLayer_12034498363969.

kernel(**inputs) takes FULL unsharded inputs, returns the FULL output.
Sharding: 8 NeuronCores = 4 batches x 2. The two cores of a batch pair
duplicate the PE-heavy phases (conv1, layernorm, attention, xz projection,
mamba branch prelims) and split the dominant vector-engine work -- the 16
selective-scan states per mamba branch -- 50/50. A single pairwise
AllReduce on the out-projection partial merges the halves; final convs are
duplicated and fc1/depthwise-conv are split by output channel half,
assembled on host.

v2 phase-5 design (calibrated on hw):
 - dA_n = exp(-n * delta) via one scalar-engine EXP per (br, dt4, state)
   (A = -n exactly for this model).
 - B/C state broadcasts via PE sel-matmuls, copied PSUM->SBUF on scalar.
 - dBu multiply + scans on Vector (scan is Vector-only; ~2.2ns/elem).
 - y = sout*C multiplies on GpSimd (keeps Vector for scans).
 - y state-sum via PE identity-matmul accumulation in PSUM (replaces the
   baseline's 84 gpsimd-issued accum-DMAs).
 - 5d gating reads the PSUM y-sum directly via Vector STT; branch combine
   via plain TT adds (reversed / unsliced views), no PSUM round-trip.
"""
import sys
sys.path.insert(0, '/opt/trn_rl_repo')
import numpy as np

import concourse.bass as bass
import concourse.mybir as mybir
import concourse.tile as tile
from concourse.masks import make_identity

f32 = mybir.dt.float32
bf16 = mybir.dt.bfloat16
FT = mybir.ActivationFunctionType
OP = mybir.AluOpType

B, C, H, W = 4, 256, 32, 32
L = H * W
DS, DC, NSL, NH, DH = 16, 4, 16, 2, 128
DI, DTR = 512, 16
P = 128
HP = (H + 2) * (W + 2)
SQ = 1.0 / float(np.sqrt(DH))


BF16_IN = {'W1T', 'QWT', 'KWT', 'VWR', 'OWT', 'INWT', 'XPWT', 'DTWT',
           'OUTWT', 'P2T', 'F1T', 'SELB', 'SELC', 'XFPAD'}


def host_prep(inp, core, nspl=8):
    import ml_dtypes
    b, s = core // 2, core % 2
    g = lambda k: np.asarray(inp[k], np.float32)
    x = g('x')
    d = {}
    x_flat = np.transpose(x, (0, 2, 1, 3)).reshape(B, C, H, W)[b]
    xfp = np.zeros((C, H + 2, W + 2), np.float32)
    xfp[:, 1:-1, 1:-1] = x_flat
    d['XFPAD'] = xfp.reshape(C, HP)
    d['XSKIP'] = x[b].reshape(C, L)[s * P:(s + 1) * P].copy()
    w1 = g('proj1_w')
    w1t = np.zeros((18, P, C), np.float32)
    for t in range(9):
        dy, dx = t // 3, t % 3
        for kt in range(2):
            w1t[t * 2 + kt] = w1[:, kt * P:(kt + 1) * P, dy, dx].T
    d['W1T'] = w1t
    d['P1B'] = g('proj1_b').reshape(C, 1)
    d['LNW'] = g('norm_w').reshape(C, 1)
    d['LNB'] = g('norm_b').reshape(C, 1)
    qw, kw, vw = g('q_w'), g('k_w'), g('v_w')
    hs_ = [s]  # this core's attention head; partial o-proj merged by AllReduce
    d['QWT'] = np.stack([qw[h * DH:(h + 1) * DH].T for h in hs_])
    d['KWT'] = np.stack([kw[h * DH:(h + 1) * DH].T for h in hs_])
    d['VWR'] = np.stack([vw[h * DH:(h + 1) * DH].T for h in hs_])
    d['QB'] = np.stack([g('q_b')[h * DH:(h + 1) * DH].reshape(DH, 1) for h in hs_])
    d['KB'] = np.stack([g('k_b')[h * DH:(h + 1) * DH].reshape(DH, 1) for h in hs_])
    d['VBR'] = np.stack([np.tile(g('v_b')[h * DH:(h + 1) * DH][None, :], (P, 1)) for h in hs_])
    d['OWT'] = np.stack([g('o_w')[:, h * DH:(h + 1) * DH].T for h in hs_])
    d['OB'] = g('o_b').reshape(C, 1) if s == 0 else np.zeros((C, 1), np.float32)
    d['INWT'] = g('in_w').T.copy()
    cwn, cbn = ['cw', 'cbw', 'csw'], ['cb', 'cbb', 'csb']
    xpn, dwn, dbn = ['xpw', 'xpbw', 'xpsw'], ['dtw', 'dtbw', 'dtsw'], ['dtb', 'dtbb', 'dtsb']
    aln, ddn = ['Alog', 'Ablog', 'Aslog'], ['D', 'Db', 'Ds']
    d['CWT'] = np.concatenate([g(cwn[i])[:, 0, :] for i in range(3)], axis=1)  # [512,12]
    d['CB'] = np.stack([g(cbn[i]) for i in range(3)], 1)
    ns0 = np.arange(s * nspl, (s + 1) * nspl) if nspl < DS else np.arange(DS)
    xps = []
    for i in range(3):
        xp = g(xpn[i]).T  # [DI, DTR+2*DS]
        xr = np.zeros_like(xp)
        xr[:, :DTR] = xp[:, :DTR]
        for j, n in enumerate(ns0):
            xr[:, DTR + j] = xp[:, DTR + n]
            xr[:, DTR + DS + j] = xp[:, DTR + DS + n]
        xps.append(xr)
    d['XPWT'] = np.stack(xps)
    d['DTWT'] = np.stack([g(dwn[i]).T for i in range(3)])
    d['DTB'] = np.stack([g(dbn[i]) for i in range(3)], 1)
    ns = np.arange(s * nspl, (s + 1) * nspl) if nspl < DS else np.arange(DS)
    asc = np.zeros((DI, 3 * nspl), np.float32)
    for i in range(3):
        A = -np.exp(g(aln[i]))
        for j, n in enumerate(ns):
            asc[:, i * nspl + j] = A[:, n]
    d['ASC'] = asc
    selb = np.zeros((3, nspl, 48, P), np.float32)
    selc = np.zeros((3, nspl, 48, P), np.float32)
    for i in range(3):
        for j, n in enumerate(ns):
            selb[i, j, DTR + n, :] = 1.0
            selc[i, j, DTR + DS + n, :] = 1.0
    d['SELB'] = selb
    d['SELC'] = selc
    # one-hot selectors for the reordered xdbl layout (state j's B at row
    # DTR+j, C at DTR+DS+j) -- identical for every branch and core.
    selj = np.zeros((48, 2 * nspl * P), np.float32)
    for j in range(nspl):
        selj[DTR + j, j * P:(j + 1) * P] = 1.0
        selj[DTR + DS + j, (nspl + j) * P:(nspl + j + 1) * P] = 1.0
    d['SELJ'] = selj
    dpp = np.stack([g(ddn[i]) for i in range(3)], 1)
    d['DPP'] = dpp if (s == 0 or nspl == DS) else np.zeros_like(dpp)
    d['OUTWT'] = g('outw').T.copy()
    d['P2T'] = g('proj2_w')[:, :, 0, 0].T.copy()
    d['P2B'] = g('proj2_b').reshape(C, 1)
    own = slice(s * P, (s + 1) * P)
    d['F1T'] = g('fc1_w')[own].T.copy()
    d['F1B'] = g('fc1_b')[own].reshape(P, 1)
    d['DWC'] = g('dw_w')[:, 0][own].reshape(P, 9)
    d['DWB'] = g('dw_b')[own].reshape(P, 1)
    for k in BF16_IN | {'SELJ'}:
        d[k] = d[k].astype(ml_dtypes.bfloat16)
    return d


NHL = 1  # heads computed per core (head split across the pair)

IN_SHAPES = [
    ('XFPAD', (C, HP)), ('XSKIP', (P, L)), ('W1T', (18, P, C)), ('P1B', (C, 1)),
    ('LNW', (C, 1)), ('LNB', (C, 1)),
    ('QWT', (NHL, C, DH)), ('KWT', (NHL, C, DH)), ('VWR', (NHL, C, DH)),
    ('QB', (NHL, DH, 1)), ('KB', (NHL, DH, 1)), ('VBR', (NHL, P, DH)),
    ('OWT', (NHL, DH, C)), ('OB', (C, 1)), ('INWT', (C, 2 * DI)),
    ('CWT', (DI, 12)), ('CB', (DI, 3)), ('XPWT', (3, DI, 48)),
    ('DTWT', (3, DTR, DI)), ('DTB', (DI, 3)),
    ('DPP', (DI, 3)), ('OUTWT', (DI, C)), ('P2T', (C, C)), ('P2B', (C, 1)),
    ('F1T', (C, P)), ('F1B', (P, 1)), ('DWC', (P, 9)), ('DWB', (P, 1)),
]


def build(nc, use_ar, group_all, nspl=8):
    din = {}
    for name, shape in IN_SHAPES + [('ASC', (DI, 3 * nspl)),
                                    ('SELB', (3, nspl, 48, P)),
                                    ('SELC', (3, nspl, 48, P)),
                                    ('SELJ', (48, 2 * nspl * P))]:
        dt_ = bf16 if (name in BF16_IN or name == 'SELJ') else f32
        din[name] = nc.dram_tensor(name, list(shape), dt_, kind="ExternalInput")
    OUTT = nc.dram_tensor('OUT', [P, L], f32, kind="ExternalOutput")
    with tile.TileContext(nc) as tc:
        prog(tc, din, OUTT, use_ar, group_all, nspl)
    return din, OUTT


def prog(tc, din, OUTT, use_ar, group_all, nspl):
    nc = tc.nc
    vengs = [nc.vector, nc.gpsimd]
    ectr = [0]

    def ve():
        ectr[0] += 1
        return vengs[ectr[0] % 2]

    A = lambda n: din[n].ap()
    NH2 = (slice(0, 512), slice(512, 1024))
    JJ = L // NSL

    def load(pool, name, view=None, tag=None):
        src = view if view is not None else A(name)
        t = pool.tile(list(src.shape), src.dtype, tag=tag or name)
        nc.sync.dma_start(t[:], src)
        return t

    def sliced(t2d):
        return t2d.rearrange("p (k j) -> p j k", k=NSL)

    def v_jk(t2d):
        return t2d.rearrange("p (j k) -> p j k", j=JJ)

    def unsliced(t2d):
        return t2d.rearrange("p (j k) -> p k j", j=JJ)

    with tc.tile_pool(name="cst", bufs=1) as cst:
        ident = cst.tile([P, P], f32, tag="ident")
        make_identity(nc, ident[:])
        identb = cst.tile([P, P], bf16, tag="identb")
        nc.scalar.copy(identb[:], ident[:])
        ones1 = cst.tile([1, P], f32, tag="ones1")
        nc.gpsimd.memset(ones1[:], 1.0)
        mean1 = cst.tile([1, P], f32, tag="mean1")
        nc.gpsimd.memset(mean1[:], 1.0 / C)
        onesk = cst.tile([P, 1], f32, tag="onesk")
        nc.gpsimd.memset(onesk[:], 1.0)
        epsb = cst.tile([P, 1], f32, tag="epsb")
        nc.gpsimd.memset(epsb[:], 1e-5)
        oneskb = cst.tile([P, 1], bf16, tag="oneskb")
        nc.gpsimd.memset(oneskb[:], 1.0)
        ones1b = cst.tile([1, P], bf16, tag="ones1b")
        nc.gpsimd.memset(ones1b[:], 1.0)
        W1T = load(cst, 'W1T', A('W1T').transpose([1, 0, 2]))
        P1B = load(cst, 'P1B', A('P1B').rearrange("(a p) o -> p a o", p=P))

        def conv3x3(getsrc, relu, dst):
            with tc.tile_pool(name="cvps", bufs=4, space="PSUM") as cps:
                for mg in range(2):
                    for nh2 in range(2):
                        pt = cps.tile([P, 512], f32, tag="convp")
                        h0 = 16 * nh2
                        k = 0
                        for t in range(9):
                            dy, dx = t // 3, t % 3
                            for kt in range(2):
                                win = getsrc(kt).rearrange("p (h w) -> p h w", h=H + 2)
                                win = win[:, dy + h0:dy + h0 + 16, dx:dx + W]
                                nc.tensor.matmul(pt[:], (W1T[:, t * 2 + kt, mg * P:(mg + 1) * P]),
                                                 (win), start=(k == 0), stop=(k == 17))
                                k += 1
                        fn = FT.Relu if relu else FT.Identity
                        nc.scalar.activation(dst(mg, nh2), pt[:], fn, bias=P1B[:, mg], scale=1.0)

        with tc.tile_pool(name="actA", bufs=1) as actA:
            xh = actA.tile([P, 4, L + DC - 1], bf16, tag="xh")
            SZ = actA.tile([P, 4, L], bf16, tag="SZ")
            comb = actA.tile([P, 4, L], bf16, tag="comb")

            # phase-5 weights prefetched before phase 1 (off the critical path)
            p5w_cm = tc.tile_pool(name="p5w", bufs=1)
            p5w = p5w_cm.__enter__()
            CWT = load(p5w, 'CWT', A('CWT').rearrange("(a p) m -> p a m", p=P))
            CBt = load(p5w, 'CB', A('CB').rearrange("(a p) m -> p a m", p=P))
            XPWT = load(p5w, 'XPWT', A('XPWT').rearrange("b (a p) m -> p b a m", p=P))
            DTWT = load(p5w, 'DTWT', A('DTWT').transpose([1, 0, 2]))
            DTB = load(p5w, 'DTB', A('DTB').rearrange("(a p) m -> p a m", p=P))
            ASC = load(p5w, 'ASC', A('ASC').rearrange("(a p) m -> p a m", p=P))
            DPP = load(p5w, 'DPP', A('DPP').rearrange("(a p) m -> p a m", p=P))
            SELJ = load(p5w, 'SELJ')

            with tc.tile_pool(name="pA", bufs=1) as pA:
                xcn = pA.tile([P, 2, L], bf16, tag="xcn")
                hsT = pA.tile([P, 2, L], bf16, tag="hsT")
                # ===== phase 1+2: conv1 + LN
                with tc.tile_pool(name="p12", bufs=1) as p12:
                    XFPAD = load(p12, 'XFPAD', A('XFPAD').rearrange("(a p) f -> p a f", p=P))
                    LNW = load(p12, 'LNW', A('LNW').rearrange("(a p) o -> p a o", p=P))
                    LNB = load(p12, 'LNB', A('LNB').rearrange("(a p) o -> p a o", p=P))
                    xc = p12.tile([P, 2, L], f32, tag="xc")
                    conv3x3(lambda kt: XFPAD[:, kt], False,
                            lambda mg, nh2: xc[:, mg, NH2[nh2]])
                    with tc.tile_pool(name="lnps", bufs=1, space="PSUM") as lps:
                        xc2 = p12.tile([P, 2, L], f32, tag="xc2")
                        for kt in range(2):
                            nc.scalar.activation(xc2[:, kt], xc[:, kt], FT.Square)
                        s1p = lps.tile([1, L], f32, tag="s1")
                        s2p = lps.tile([1, L], f32, tag="s2")
                        for nh2 in range(2):
                            for kt in range(2):
                                nc.tensor.matmul(s1p[:, NH2[nh2]], (onesk[:]), (xc[:, kt, NH2[nh2]]),
                                                 start=(kt == 0), stop=(kt == 1))
                                nc.tensor.matmul(s2p[:, NH2[nh2]], (onesk[:]), (xc2[:, kt, NH2[nh2]]),
                                                 start=(kt == 0), stop=(kt == 1))
                        s12 = p12.tile([1, 2, L], f32, tag="s12")
                        nc.vector.tensor_copy(s12[:, 0], s1p[:])
                        nc.vector.tensor_copy(s12[:, 1], s2p[:])
                        mrep = lps.tile([P, L], f32, tag="mrep")
                        vrep = lps.tile([P, L], f32, tag="vrep")
                        for nh2 in range(2):
                            nc.tensor.matmul(mrep[:, NH2[nh2]], (mean1[:]), (s12[:, 0, NH2[nh2]]),
                                             start=True, stop=True)
                            nc.tensor.matmul(vrep[:, NH2[nh2]], (mean1[:]), (s12[:, 1, NH2[nh2]]),
                                             start=True, stop=True)
                        mu2 = p12.tile([P, L], f32, tag="mu2")
                        nc.scalar.activation(mu2[:], mrep[:], FT.Square)
                        varr = p12.tile([P, L], f32, tag="varr")
                        nc.vector.tensor_tensor(varr[:], vrep[:], mu2[:], OP.subtract)
                        # 1/sqrt(var+eps) = exp(-0.5*ln(var+eps)): stays in the
                        # exp/ln table set and avoids the 6.5us single-rate
                        # vector reciprocal.
                        stdt = p12.tile([P, L], f32, tag="stdt")
                        nc.scalar.activation(stdt[:], varr[:], FT.Ln, bias=epsb[:])
                        inv = p12.tile([P, L], f32, tag="inv")
                        nc.scalar.activation(inv[:], stdt[:], FT.Exp, scale=-0.5)
                        for kt in range(2):
                            t1 = p12.tile([P, L], f32, tag="lnt1")
                            nc.vector.tensor_tensor(t1[:], xc[:, kt], mrep[:], OP.subtract)
                            t2 = p12.tile([P, L], f32, tag="lnt2")
                            nc.gpsimd.tensor_tensor(t2[:], t1[:], inv[:], OP.mult)
                            nc.scalar.activation(xcn[:, kt], t2[:], FT.Identity,
                                                 bias=LNB[:, kt], scale=LNW[:, kt])

                # ===== phase 3: attention (head split across the core pair;
                # partial o-proj merged with a pairwise AllReduce)
                with tc.tile_pool(name="p3", bufs=2) as p3, \
                     tc.tile_pool(name="ardA", bufs=1, space="DRAM") as ardA:
                    QWT = load(p3, 'QWT', A('QWT').rearrange("h (a p) m -> p h a m", p=P))
                    KWT = load(p3, 'KWT', A('KWT').rearrange("h (a p) m -> p h a m", p=P))
                    VWR = load(p3, 'VWR', A('VWR').rearrange("h (a p) m -> p h a m", p=P))
                    QB = load(p3, 'QB', A('QB').transpose([1, 0, 2]))
                    KB = load(p3, 'KB', A('KB').transpose([1, 0, 2]))
                    VBR = load(p3, 'VBR', A('VBR').transpose([1, 0, 2]))
                    OWT = load(p3, 'OWT', A('OWT').transpose([1, 0, 2]))
                    OB = load(p3, 'OB', A('OB').rearrange("(a p) o -> p a o", p=P))
                    Osb = p3.tile([P, 2, L], bf16, tag="Osb")
                    for h in range(NHL):
                        with tc.tile_pool(name="qkps", bufs=2, space="PSUM") as qps:
                            Qp = qps.tile([DH, L], f32, tag="qkp")
                            Kp = qps.tile([DH, L], f32, tag="qkp")
                            for nh2 in range(2):
                                for kt in range(2):
                                    nc.tensor.matmul(Qp[:, NH2[nh2]], (QWT[:, h, kt]),
                                                     (xcn[:, kt, NH2[nh2]]), start=(kt == 0), stop=(kt == 1))
                                    nc.tensor.matmul(Kp[:, NH2[nh2]], (KWT[:, h, kt]),
                                                     (xcn[:, kt, NH2[nh2]]), start=(kt == 0), stop=(kt == 1))
                            Q = p3.tile([DH, L], bf16, tag="Q")
                            Kt = p3.tile([DH, L], bf16, tag="K")
                            nc.scalar.activation(Q[:], Qp[:], FT.Identity, bias=QB[:, h])
                            nc.scalar.activation(Kt[:], Kp[:], FT.Identity, bias=KB[:, h])
                        Vt = p3.tile([P, 8, DH], bf16, tag="Vt")
                        with tc.tile_pool(name="vps", bufs=2, space="PSUM") as vps:
                            for mgr in range(8):
                                vp = vps.tile([P, DH], f32, tag="vp")
                                for kt in range(2):
                                    nc.tensor.matmul(vp[:], (xcn[:, kt, mgr * P:(mgr + 1) * P]),
                                                     (VWR[:, h, kt]), start=(kt == 0), stop=(kt == 1))
                                nc.vector.tensor_tensor(Vt[:, mgr], vp[:], VBR[:, h], OP.add)
                        expt = p3.tile([P, 8, L], bf16, tag="expt")
                        den = p3.tile([1, 2, L], f32, tag="den")
                        with tc.tile_pool(name="sps", bufs=3, space="PSUM") as spsp, \
                             tc.tile_pool(name="dps", bufs=1, space="PSUM") as dpsp:
                            denp = dpsp.tile([1, L], f32, tag="denp")
                            for nkt in range(8):
                                sp = spsp.tile([P, L], f32, tag="sp")
                                for nh2 in range(2):
                                    nc.tensor.matmul(sp[:, NH2[nh2]], (Kt[:, nkt * P:(nkt + 1) * P]),
                                                     (Q[:, NH2[nh2]]), start=True, stop=True)
                                nc.scalar.activation(expt[:, nkt], sp[:], FT.Exp, scale=SQ)
                                for nh2 in range(2):
                                    nc.tensor.matmul(denp[:, NH2[nh2]], (oneskb[:]),
                                                     (expt[:, nkt, NH2[nh2]]),
                                                     start=(nkt == 0), stop=(nkt == 7))
                            nc.scalar.activation(den[:, 0], denp[:], FT.Ln)
                        nc.scalar.activation(den[:, 1], den[:, 0], FT.Exp, scale=-1.0)
                        with tc.tile_pool(name="pvps", bufs=1, space="PSUM") as pvps:
                            denir_p = pvps.tile([P, L], f32, tag="denir")
                            for nh2 in range(2):
                                nc.tensor.matmul(denir_p[:, NH2[nh2]], (ones1[:]),
                                                 (den[:, 1, NH2[nh2]]), start=True, stop=True)
                            denir = p3.tile([P, L], f32, tag="denirs")
                            nc.vector.tensor_copy(denir[:], denir_p[:])
                            attp = pvps.tile([DH, L], f32, tag="attp")
                            for nkt in range(8):
                                for nh2 in range(2):
                                    nc.tensor.matmul(attp[:, NH2[nh2]], (Vt[:, nkt]),
                                                     (expt[:, nkt, NH2[nh2]]),
                                                     start=(nkt == 0), stop=(nkt == 7))
                            att = p3.tile([DH, L], bf16, tag="att")
                            nc.vector.tensor_tensor(att[:], attp[:], denir[:], OP.mult)
                            Oph = pvps.tile([P, 2, L], f32, tag="oph")
                            for mg in range(2):
                                for nh2 in range(2):
                                    nc.tensor.matmul(Oph[:, mg, NH2[nh2]], (OWT[:, h, mg * P:(mg + 1) * P]),
                                                     (att[:, NH2[nh2]]), start=True, stop=True)
                            for mg in range(2):
                                nc.scalar.activation(Osb[:, mg], Oph[:, mg], FT.Identity, bias=OB[:, mg])
                    aOin = ardA.tile([C, L], bf16, tag="aOin")
                    aOout = ardA.tile([C, L], bf16, tag="aOout")
                    nc.sync.dma_start(aOin[:].rearrange("(a p) l -> p a l", p=P), Osb[:])
                    if use_ar:
                        nc.gpsimd.collective_compute("AllReduce", OP.add, replica_groups=group_all,
                                                     ins=[aOin.opt()], outs=[aOout.opt()])
                        nc.sync.dma_start(Osb[:], aOout[:].rearrange("(a p) l -> p a l", p=P))
                    with tc.tile_pool(name="trps", bufs=4, space="PSUM") as tps:
                        for q in range(4):
                            for mg in range(2):
                                for cg in range(2):
                                    tp = tps.tile([P, P], bf16, tag="trp")
                                    src = Osb[:, mg].rearrange("p (a b) -> p a b", b=4)[:, :, q]
                                    nc.tensor.transpose(tp[:], src[:, cg * P:(cg + 1) * P], identb[:])
                                    nc.vector.tensor_copy(hsT[:, cg, q * 256 + mg * P: q * 256 + (mg + 1) * P], tp[:])

                # ===== phase 4: xz projection
                for dt4 in range(4):
                    nc.gpsimd.memset(xh[:, dt4, 0:DC - 1], 0.0)
                with tc.tile_pool(name="p4", bufs=1) as p4:
                    INWT = load(p4, 'INWT', A('INWT').rearrange("(a p) m -> p a m", p=P))
                    with tc.tile_pool(name="xzps", bufs=4, space="PSUM") as xps:
                        for mg in range(8):
                            pt = xps.tile([P, L], f32, tag="xzp")
                            for nh2 in range(2):
                                for kt in range(2):
                                    nc.tensor.matmul(pt[:, NH2[nh2]], (INWT[:, kt, mg * P:(mg + 1) * P]),
                                                     (hsT[:, kt, NH2[nh2]]), start=(kt == 0), stop=(kt == 1))
                            if mg < 4:
                                nc.vector.tensor_copy(xh[:, mg, DC - 1:], pt[:])
                            else:
                                nc.scalar.activation(SZ[:, mg - 4], pt[:], FT.Silu)

            # ===== phase 5: mamba branches (v2; pA closed -> xcn/hsT freed)
            if True:
                with tc.tile_pool(name="brt", bufs=1) as bp, \
                     tc.tile_pool(name="brtmp", bufs=1) as btmp:
                    xms, xdbls, deltas, dus = {}, {}, {}, {}
                    for br in range(3):
                        xms[br] = bp.tile([P, 4, L], bf16, tag=f"xm{br}", name=f"xm{br}")
                        xdbls[br] = bp.tile([48, L], bf16, tag=f"xdbl{br}", name=f"xdbl{br}")
                        deltas[br] = bp.tile([P, 4, L], bf16, tag=f"delta{br}", name=f"delta{br}")
                        dus[br] = bp.tile([P, 4, L], bf16, tag=f"du{br}", name=f"du{br}")
                    # --- 5a: conv1d + silu for all branches
                    with tc.tile_pool(name="xpadp", bufs=1) as xpp, \
                         tc.tile_pool(name="brps", bufs=4, space="PSUM") as bps:
                        for br in range(3):
                            xm = xms[br]
                            if br == 0:
                                xpadv = xh
                            else:
                                xpadv = xpp.tile([P, 4, L + DC - 1], bf16, tag="xpad")
                                for dt4 in range(4):
                                    nc.gpsimd.memset(xpadv[:, dt4, 0:DC - 1], 0.0)
                                    if br == 1:
                                        nc.vector.tensor_copy(xpadv[:, dt4, DC - 1:], xh[:, dt4, DC - 1:][:, ::-1])
                                    else:
                                        nc.vector.tensor_copy(v_jk(xpadv[:, dt4, DC - 1:]), sliced(xh[:, dt4, DC - 1:]))
                            dg = btmp.tile([P, DC, P], bf16, tag="cdiag")
                            for dt4 in range(4):
                                for j in range(DC):
                                    nc.vector.tensor_scalar_mul(dg[:, j], identb[:], CWT[:, dt4, br * DC + j:br * DC + j + 1])
                                pt = bps.tile([P, L], f32, tag="cvp")
                                for nh2 in range(2):
                                    for j in range(DC):
                                        nc.tensor.matmul(pt[:, NH2[nh2]], (dg[:, j]),
                                                         (xpadv[:, dt4, j + nh2 * 512: j + nh2 * 512 + 512]),
                                                         start=(j == 0), stop=(j == DC - 1))
                                nc.scalar.activation(xm[:, dt4], pt[:], FT.Silu,
                                                     bias=CBt[:, dt4, br:br + 1])
                    # --- 5b: x_dbl + softplus + du for all branches
                    with tc.tile_pool(name="xdpp", bufs=2, space="PSUM") as xdpp, \
                         tc.tile_pool(name="dtpp", bufs=2, space="PSUM") as dtpp:
                        for br in range(3):
                            xm, xdbl, delta, du = xms[br], xdbls[br], deltas[br], dus[br]
                            xdp = xdpp.tile([48, L], f32, tag="xdp")
                            for nh2 in range(2):
                                for kt in range(4):
                                    nc.tensor.matmul(xdp[:, NH2[nh2]], (XPWT[:, br, kt]),
                                                     (xm[:, kt, NH2[nh2]]), start=(kt == 0), stop=(kt == 3))
                            nc.vector.tensor_copy(xdbl[:], xdp[:])
                            for dt4 in range(4):
                                dtp = dtpp.tile([P, L], f32, tag="dtp")
                                for nh2 in range(2):
                                    nc.tensor.matmul(dtp[:, NH2[nh2]], (DTWT[:, br, dt4 * P:(dt4 + 1) * P]),
                                                     (xdbl[:DTR, NH2[nh2]]), start=True, stop=True)
                                spe = btmp.tile([P, L], bf16, tag="yf")
                                nc.scalar.activation(spe[:], dtp[:], FT.Exp,
                                                     bias=DTB[:, dt4, br:br + 1])
                                nc.scalar.activation(delta[:, dt4], spe[:], FT.Ln, bias=1.0)
                            nc.vector.tensor_tensor(
                                du[:].rearrange("p a b -> p (a b)"),
                                delta[:].rearrange("p a b -> p (a b)"),
                                xm[:].rearrange("p a b -> p (a b)"), OP.mult)
                    # --- 5c v3: JIT PSUM broadcasts + 4-state chained scans.
                    # The scan chains 4 states in one instruction; zeroing the
                    # first dA column of every state segment makes the chaining
                    # exact (s_0 = dBu_0 regardless of carried state).
                    with tc.tile_pool(name="bcsb", bufs=2) as bcp, \
                         tc.tile_pool(name="sctA", bufs=2) as scpA, \
                         tc.tile_pool(name="sctB", bufs=2) as scpB, \
                         tc.tile_pool(name="sctS", bufs=2) as scpS, \
                         tc.tile_pool(name="bcps", bufs=2, space="PSUM") as bcps, \
                         tc.tile_pool(name="ysps", bufs=1, space="PSUM") as ysps:
                        # pre-zero column 0 of both dA4 buffers once; the exp
                        # activations write only [:, 1:], so the zeros persist
                        # and make the 4-state scan chaining exact.
                        for _ in range(2):
                            t0_ = scpA.tile([P, 4, L], bf16, tag="dA4")
                            nc.vector.memset(t0_[:, :, 0:1], 0.0)
                        for br in range(3):
                            xm, xdbl, delta, du = xms[br], xdbls[br], deltas[br], dus[br]
                            # prebuild B/C broadcast tiles per state-half (shared
                            # across the dt4 loop; bufs=2 pipelines half1 builds
                            # under half0 scans)
                            bch = {}
                            for half in range(nspl // 4):
                                j0 = half * 4
                                bc4 = bcp.tile([P, 4, 2, L], bf16, tag="bc4")
                                bch[half] = bc4
                                for jj in range(4):
                                    j = j0 + jj
                                    brp = bcps.tile([P, L], f32, tag="bcj")
                                    for nh2 in range(2):
                                        nc.tensor.matmul(brp[:, NH2[nh2]], (SELJ[:, j * P:(j + 1) * P]),
                                                         (xdbl[:, NH2[nh2]]), start=True, stop=True)
                                    nc.scalar.copy(bc4[:, jj, 0], brp[:])
                                    crp = bcps.tile([P, L], f32, tag="bcj")
                                    for nh2 in range(2):
                                        nc.tensor.matmul(crp[:, NH2[nh2]], (SELJ[:, (nspl + j) * P:(nspl + j + 1) * P]),
                                                         (xdbl[:, NH2[nh2]]), start=True, stop=True)
                                    nc.scalar.copy(bc4[:, jj, 1], crp[:])
                            for dt4 in range(4):
                                yaccp = ysps.tile([P, L], f32, tag="yacc")
                                for half in range(nspl // 4):
                                    j0 = half * 4
                                    bc4 = bch[half]
                                    dA4 = scpA.tile([P, 4, L], bf16, tag="dA4")
                                    for jj in range(4):
                                        j = j0 + jj
                                        nc.scalar.activation(dA4[:, jj, 1:], delta[:, dt4, 1:], FT.Exp,
                                                             scale=ASC[:, dt4, br * nspl + j:br * nspl + j + 1])
                                    dBu4 = scpB.tile([P, 4, L], bf16, tag="dBu4")
                                    for jj in range(4):
                                        # jj=3 on gpsimd balances the per-half
                                        # load (vector: 3 TT + scan, gpsimd:
                                        # ypk + 1 TT)
                                        eng = nc.gpsimd if jj == 3 else nc.vector
                                        eng.tensor_tensor(dBu4[:, jj], du[:, dt4], bc4[:, jj, 0], OP.mult)
                                    sout = scpS.tile([P, 4, L], bf16, tag="sout")
                                    nc.vector.tensor_tensor_scan(
                                        sout[:].rearrange("p a b -> p (a b)"),
                                        dA4[:].rearrange("p a b -> p (a b)"),
                                        dBu4[:].rearrange("p a b -> p (a b)"),
                                        0.0, OP.mult, OP.add)
                                    # ypk = sout * C on gpsimd, written over the consumed dBu4
                                    nc.gpsimd.tensor_tensor(
                                        dBu4[:], sout[:], bc4[:, :, 1], OP.mult)
                                    for jj in range(4):
                                        j = j0 + jj
                                        for nh2 in range(2):
                                            nc.tensor.matmul(yaccp[:, NH2[nh2]], (identb[:]),
                                                             (dBu4[:, jj, NH2[nh2]]),
                                                             start=(j == 0), stop=(j == nspl - 1))
                                # --- 5d for this (br, dt4)
                                yf = btmp.tile([P, L], bf16, tag="yf")
                                nc.vector.scalar_tensor_tensor(yf[:], xm[:, dt4], DPP[:, dt4, br:br + 1],
                                                               yaccp[:], OP.mult, OP.add)
                                if br == 0:
                                    nc.vector.tensor_tensor(comb[:, dt4], yf[:], SZ[:, dt4], OP.mult)
                                elif br == 1:
                                    yg = btmp.tile([P, L], bf16, tag="yg")
                                    nc.gpsimd.tensor_tensor(yg[:], yf[:], SZ[:, dt4][:, ::-1], OP.mult)
                                    nc.vector.tensor_tensor(comb[:, dt4], comb[:, dt4], yg[:][:, ::-1], OP.add)
                                else:
                                    yg = btmp.tile([P, L], bf16, tag="yg")
                                    nc.gpsimd.tensor_tensor(v_jk(yg[:]), v_jk(yf[:]), sliced(SZ[:, dt4]), OP.mult)
                                    nc.vector.tensor_tensor(comb[:, dt4], comb[:, dt4], unsliced(yg[:]), OP.add)

            p5w_cm.__exit__(None, None, None)
            # ===== phases 6+7 share a tail pool (p7 weights prefetch first)
            with tc.tile_pool(name="p7", bufs=1) as p7:
                Mfull = p7.tile([P, 2, L], bf16, tag="Mfull")
                P2T = load(p7, 'P2T', A('P2T').rearrange("(a p) m -> p a m", p=P))
                P2B = load(p7, 'P2B', A('P2B').rearrange("(a p) o -> p a o", p=P))
                F1T = load(p7, 'F1T', A('F1T').rearrange("(a p) m -> p a m", p=P))
                F1B = load(p7, 'F1B')
                DWC = load(p7, 'DWC')
                DWB = load(p7, 'DWB')
                XSKIP = load(p7, 'XSKIP')
                # ===== phase 6: out projection + AllReduce
                with tc.tile_pool(name="p6", bufs=1) as p6, \
                     tc.tile_pool(name="mps", bufs=2, space="PSUM") as mps, \
                     tc.tile_pool(name="ardram", bufs=1, space="DRAM") as ard:
                    OUTWT = load(p6, 'OUTWT', A('OUTWT').rearrange("(a p) m -> p a m", p=P))
                    Mpart = p6.tile([P, 2, L], bf16, tag="mpart")
                    for mg in range(2):
                        mp = mps.tile([P, L], f32, tag="mp")
                        for nh2 in range(2):
                            for kt in range(4):
                                nc.tensor.matmul(mp[:, NH2[nh2]], (OUTWT[:, kt, mg * P:(mg + 1) * P]),
                                                 (comb[:, kt, NH2[nh2]]), start=(kt == 0), stop=(kt == 3))
                        nc.scalar.copy(Mpart[:, mg], mp[:])
                    bin_ = ard.tile([C, L], bf16, tag="arin")
                    bout = ard.tile([C, L], bf16, tag="arout")
                    nc.sync.dma_start(bin_[:].rearrange("(a p) l -> p a l", p=P), Mpart[:])
                    if use_ar:
                        nc.gpsimd.collective_compute("AllReduce", OP.add, replica_groups=group_all,
                                                     ins=[bin_.opt()], outs=[bout.opt()])
                        nc.sync.dma_start(Mfull[:], bout[:].rearrange("(a p) l -> p a l", p=P))
                    else:
                        nc.sync.dma_start(Mfull[:], bin_[:].rearrange("(a p) l -> p a l", p=P))

                # ===== phase 7: conv1#2, conv2, fc1, dw + residual
                mpad = p7.tile([P, 2, HP], bf16, tag="mpad")
                for mg in range(2):
                    nc.gpsimd.memset(mpad[:, mg], 0.0)
                    dst = mpad[:, mg].rearrange("p (h w) -> p h w", h=H + 2)[:, 1:H + 1, 1:W + 1]
                    nc.vector.tensor_copy(dst, Mfull[:, mg].rearrange("p (h w) -> p h w", h=H))
                c1 = p7.tile([P, 2, L], bf16, tag="c1")
                conv3x3(lambda kt: mpad[:, kt], True,
                        lambda mg, nh2: c1[:, mg, NH2[nh2]])
                c2 = p7.tile([P, 2, L], bf16, tag="c2")
                with tc.tile_pool(name="c2ps", bufs=2, space="PSUM") as cps:
                    for mg in range(2):
                        for nh2 in range(2):
                            pt = cps.tile([P, 512], f32, tag="c2p")
                            for kt in range(2):
                                nc.tensor.matmul(pt[:], (P2T[:, kt, mg * P:(mg + 1) * P]),
                                                 (c1[:, kt, NH2[nh2]]), start=(kt == 0), stop=(kt == 1))
                            nc.scalar.activation(c2[:, mg, NH2[nh2]], pt[:], FT.Relu, bias=P2B[:, mg])
                    xfpad = p7.tile([P, HP], bf16, tag="xfpad")
                    nc.gpsimd.memset(xfpad[:], 0.0)
                    for nh2 in range(2):
                        pt = cps.tile([P, 512], f32, tag="fcp")
                        for kt in range(2):
                            nc.tensor.matmul(pt[:], (F1T[:, kt]), (c2[:, kt, NH2[nh2]]),
                                             start=(kt == 0), stop=(kt == 1))
                        dstv = xfpad[:].rearrange("p (h w) -> p h w", h=H + 2)[:, 1 + 16 * nh2:17 + 16 * nh2, 1:W + 1]
                        nc.scalar.activation(dstv, pt[:].rearrange("p (h w) -> p h w", h=16),
                                             FT.Identity, bias=F1B[:])
                    dwg = p7.tile([P, 9, P], bf16, tag="dwg")
                    for t in range(9):
                        nc.vector.tensor_scalar_mul(dwg[:, t], identb[:], DWC[:, t:t + 1])
                    outsb = p7.tile([P, L], f32, tag="outsb")
                    for nh2 in range(2):
                        pt = cps.tile([P, 512], f32, tag="dwp")
                        h0 = 16 * nh2
                        for t in range(9):
                            dy, dx = t // 3, t % 3
                            win = xfpad[:].rearrange("p (h w) -> p h w", h=H + 2)
                            win = win[:, dy + h0:dy + h0 + 16, dx:dx + W]
                            nc.tensor.matmul(pt[:], (dwg[:, t]), (win), start=(t == 0), stop=(t == 8))
                        dwt = p7.tile([P, 512], f32, tag="dwt")
                        nc.scalar.activation(dwt[:], pt[:], FT.Identity, bias=DWB[:])
                        nc.vector.tensor_tensor(outsb[:, NH2[nh2]], dwt[:],
                                                XSKIP[:, NH2[nh2]], OP.add)
                    nc.sync.dma_start(OUTT.ap(), outsb[:])


_CACHE = {}


def _build():
    if 'nc' in _CACHE:
        return
    from concourse import bacc
    nc = bacc.Bacc(target_bir_lowering=False)
    group = [[0, 1], [2, 3], [4, 5], [6, 7]]
    build(nc, use_ar=True, group_all=group, nspl=8)
    nc.compile()
    _CACHE['nc'] = nc


def kernel(**inputs):
    _build()
    from concourse.bass_utils import run_bass_kernel_spmd
    nc = _CACHE['nc']
    in_maps = [host_prep(inputs, core, nspl=8) for core in range(8)]
    res = run_bass_kernel_spmd(nc, in_maps, core_ids=list(range(8)))
    out = np.zeros((B, C, H * W), np.float32)
    for core in range(8):
        b, s = core // 2, core % 2
        out[b, s * 128:(s + 1) * 128] = res.results[core]['OUT']
    return out.reshape(B, C, H, W)



# revision 48
# speedup vs baseline: 1.0039x; 1.0039x over previous
"""Self-contained Trainium2 Bass kernel for nn_Att_MambaLayer_12034498363969.

kernel(**inputs) takes FULL unsharded inputs, returns the FULL output.
Sharding: 8 NeuronCores = 4 batches x 2. The two cores of a batch pair
duplicate the PE-heavy phases (conv1, layernorm, attention, xz projection,
mamba branch prelims) and split the dominant vector-engine work -- the 16
selective-scan states per mamba branch -- 50/50. A single pairwise
AllReduce on the out-projection partial merges the halves; final convs are
duplicated and fc1/depthwise-conv are split by output channel half,
assembled on host.

v2 phase-5 design (calibrated on hw):
 - dA_n = exp(-n * delta) via one scalar-engine EXP per (br, dt4, state)
   (A = -n exactly for this model).
 - B/C state broadcasts via PE sel-matmuls, copied PSUM->SBUF on scalar.
 - dBu multiply + scans on Vector (scan is Vector-only; ~2.2ns/elem).
 - y = sout*C multiplies on GpSimd (keeps Vector for scans).
 - y state-sum via PE identity-matmul accumulation in PSUM (replaces the
   baseline's 84 gpsimd-issued accum-DMAs).
 - 5d gating reads the PSUM y-sum directly via Vector STT; branch combine
   via plain TT adds (reversed / unsliced views), no PSUM round-trip.
"""
import sys
sys.path.insert(0, '/opt/trn_rl_repo')
import numpy as np

import concourse.bass as bass
import concourse.mybir as mybir
import concourse.tile as tile
from concourse.masks import make_identity

f32 = mybir.dt.float32
bf16 = mybir.dt.bfloat16
FT = mybir.ActivationFunctionType
OP = mybir.AluOpType

B, C, H, W = 4, 256, 32, 32
L = H * W
DS, DC, NSL, NH, DH = 16, 4, 16, 2, 128
DI, DTR = 512, 16
P = 128
HP = (H + 2) * (W + 2)
SQ = 1.0 / float(np.sqrt(DH))


BF16_IN = {'W1T', 'QWT', 'KWT', 'VWR', 'OWT', 'INWT', 'XPWT', 'DTWT',
           'OUTWT', 'P2T', 'F1T', 'SELB', 'SELC', 'XFPAD'}


def host_prep(inp, core, nspl=8):
    import ml_dtypes
    b, s = core // 2, core % 2
    g = lambda k: np.asarray(inp[k], np.float32)
    x = g('x')
    d = {}
    x_flat = np.transpose(x, (0, 2, 1, 3)).reshape(B, C, H, W)[b]
    xfp = np.zeros((C, H + 2, W + 2), np.float32)
    xfp[:, 1:-1, 1:-1] = x_flat
    d['XFPAD'] = xfp.reshape(C, HP)
    d['XSKIP'] = x[b].reshape(C, L)[s * P:(s + 1) * P].copy()
    w1 = g('proj1_w')
    w1t = np.zeros((18, P, C), np.float32)
    for t in range(9):
        dy, dx = t // 3, t % 3
        for kt in range(2):
            w1t[t * 2 + kt] = w1[:, kt * P:(kt + 1) * P, dy, dx].T
    d['W1T'] = w1t
    d['P1B'] = g('proj1_b').reshape(C, 1)
    d['LNW'] = g('norm_w').reshape(C, 1)
    d['LNB'] = g('norm_b').reshape(C, 1)
    qw, kw, vw = g('q_w'), g('k_w'), g('v_w')
    hs_ = [s]  # this core's attention head; partial o-proj merged by AllReduce
    d['QWT'] = np.stack([qw[h * DH:(h + 1) * DH].T for h in hs_])
    d['KWT'] = np.stack([kw[h * DH:(h + 1) * DH].T for h in hs_])
    d['VWR'] = np.stack([vw[h * DH:(h + 1) * DH].T for h in hs_])
    d['QB'] = np.stack([g('q_b')[h * DH:(h + 1) * DH].reshape(DH, 1) for h in hs_])
    d['KB'] = np.stack([g('k_b')[h * DH:(h + 1) * DH].reshape(DH, 1) for h in hs_])
    d['VBR'] = np.stack([np.tile(g('v_b')[h * DH:(h + 1) * DH][None, :], (P, 1)) for h in hs_])
    d['OWT'] = np.stack([g('o_w')[:, h * DH:(h + 1) * DH].T for h in hs_])
    d['OB'] = g('o_b').reshape(C, 1) if s == 0 else np.zeros((C, 1), np.float32)
    d['INWT'] = g('in_w').T.copy()
    cwn, cbn = ['cw', 'cbw', 'csw'], ['cb', 'cbb', 'csb']
    xpn, dwn, dbn = ['xpw', 'xpbw', 'xpsw'], ['dtw', 'dtbw', 'dtsw'], ['dtb', 'dtbb', 'dtsb']
    aln, ddn = ['Alog', 'Ablog', 'Aslog'], ['D', 'Db', 'Ds']
    d['CWT'] = np.concatenate([g(cwn[i])[:, 0, :] for i in range(3)], axis=1)  # [512,12]
    d['CB'] = np.stack([g(cbn[i]) for i in range(3)], 1)
    ns0 = np.arange(s * nspl, (s + 1) * nspl) if nspl < DS else np.arange(DS)
    xps = []
    for i in range(3):
        xp = g(xpn[i]).T  # [DI, DTR+2*DS]
        xr = np.zeros_like(xp)
        xr[:, :DTR] = xp[:, :DTR]
        for j, n in enumerate(ns0):
            xr[:, DTR + j] = xp[:, DTR + n]
            xr[:, DTR + DS + j] = xp[:, DTR + DS + n]
        xps.append(xr)
    d['XPWT'] = np.stack(xps)
    d['DTWT'] = np.stack([g(dwn[i]).T for i in range(3)])
    d['DTB'] = np.stack([g(dbn[i]) for i in range(3)], 1)
    ns = np.arange(s * nspl, (s + 1) * nspl) if nspl < DS else np.arange(DS)
    asc = np.zeros((DI, 3 * nspl), np.float32)
    for i in range(3):
        A = -np.exp(g(aln[i]))
        for j, n in enumerate(ns):
            asc[:, i * nspl + j] = A[:, n]
    d['ASC'] = asc
    selb = np.zeros((3, nspl, 48, P), np.float32)
    selc = np.zeros((3, nspl, 48, P), np.float32)
    for i in range(3):
        for j, n in enumerate(ns):
            selb[i, j, DTR + n, :] = 1.0
            selc[i, j, DTR + DS + n, :] = 1.0
    d['SELB'] = selb
    d['SELC'] = selc
    # one-hot selectors for the reordered xdbl layout (state j's B at row
    # DTR+j, C at DTR+DS+j) -- identical for every branch and core.
    selj = np.zeros((48, 2 * nspl * P), np.float32)
    for j in range(nspl):
        selj[DTR + j, j * P:(j + 1) * P] = 1.0
        selj[DTR + DS + j, (nspl + j) * P:(nspl + j + 1) * P] = 1.0
    d['SELJ'] = selj
    dpp = np.stack([g(ddn[i]) for i in range(3)], 1)
    d['DPP'] = dpp if (s == 0 or nspl == DS) else np.zeros_like(dpp)
    d['OUTWT'] = g('outw').T.copy()
    d['P2T'] = g('proj2_w')[:, :, 0, 0].T.copy()
    d['P2B'] = g('proj2_b').reshape(C, 1)
    own = slice(s * P, (s + 1) * P)
    d['F1T'] = g('fc1_w')[own].T.copy()
    d['F1B'] = g('fc1_b')[own].reshape(P, 1)
    d['DWC'] = g('dw_w')[:, 0][own].reshape(P, 9)
    d['DWB'] = g('dw_b')[own].reshape(P, 1)
    for k in BF16_IN | {'SELJ'}:
        d[k] = d[k].astype(ml_dtypes.bfloat16)
    return d


NHL = 1  # heads computed per core (head split across the pair)

IN_SHAPES = [
    ('XFPAD', (C, HP)), ('XSKIP', (P, L)), ('W1T', (18, P, C)), ('P1B', (C, 1)),
    ('LNW', (C, 1)), ('LNB', (C, 1)),
    ('QWT', (NHL, C, DH)), ('KWT', (NHL, C, DH)), ('VWR', (NHL, C, DH)),
    ('QB', (NHL, DH, 1)), ('KB', (NHL, DH, 1)), ('VBR', (NHL, P, DH)),
    ('OWT', (NHL, DH, C)), ('OB', (C, 1)), ('INWT', (C, 2 * DI)),
    ('CWT', (DI, 12)), ('CB', (DI, 3)), ('XPWT', (3, DI, 48)),
    ('DTWT', (3, DTR, DI)), ('DTB', (DI, 3)),
    ('DPP', (DI, 3)), ('OUTWT', (DI, C)), ('P2T', (C, C)), ('P2B', (C, 1)),
    ('F1T', (C, P)), ('F1B', (P, 1)), ('DWC', (P, 9)), ('DWB', (P, 1)),
]


def build(nc, use_ar, group_all, nspl=8):
    din = {}
    for name, shape in IN_SHAPES + [('ASC', (DI, 3 * nspl)),
                                    ('SELB', (3, nspl, 48, P)),
                                    ('SELC', (3, nspl, 48, P)),
                                    ('SELJ', (48, 2 * nspl * P))]:
        dt_ = bf16 if (name in BF16_IN or name == 'SELJ') else f32
        din[name] = nc.dram_tensor(name, list(shape), dt_, kind="ExternalInput")
    OUTT = nc.dram_tensor('OUT', [P, L], f32, kind="ExternalOutput")
    with tile.TileContext(nc) as tc:
        prog(tc, din, OUTT, use_ar, group_all, nspl)
    return din, OUTT


def prog(tc, din, OUTT, use_ar, group_all, nspl):
    nc = tc.nc
    vengs = [nc.vector, nc.gpsimd]
    ectr = [0]

    def ve():
        ectr[0] += 1
        return vengs[ectr[0] % 2]

    A = lambda n: din[n].ap()
    NH2 = (slice(0, 512), slice(512, 1024))
    JJ = L // NSL

    def load(pool, name, view=None, tag=None):
        src = view if view is not None else A(name)
        t = pool.tile(list(src.shape), src.dtype, tag=tag or name)
        nc.sync.dma_start(t[:], src)
        return t

    def sliced(t2d):
        return t2d.rearrange("p (k j) -> p j k", k=NSL)

    def v_jk(t2d):
        return t2d.rearrange("p (j k) -> p j k", j=JJ)

    def unsliced(t2d):
        return t2d.rearrange("p (j k) -> p k j", j=JJ)

    with tc.tile_pool(name="cst", bufs=1) as cst:
        ident = cst.tile([P, P], f32, tag="ident")
        make_identity(nc, ident[:])
        identb = cst.tile([P, P], bf16, tag="identb")
        nc.scalar.copy(identb[:], ident[:])
        ones1 = cst.tile([1, P], f32, tag="ones1")
        nc.gpsimd.memset(ones1[:], 1.0)
        mean1 = cst.tile([1, P], f32, tag="mean1")
        nc.gpsimd.memset(mean1[:], 1.0 / C)
        onesk = cst.tile([P, 1], f32, tag="onesk")
        nc.gpsimd.memset(onesk[:], 1.0)
        epsb = cst.tile([P, 1], f32, tag="epsb")
        nc.gpsimd.memset(epsb[:], 1e-5)
        oneskb = cst.tile([P, 1], bf16, tag="oneskb")
        nc.gpsimd.memset(oneskb[:], 1.0)
        ones1b = cst.tile([1, P], bf16, tag="ones1b")
        nc.gpsimd.memset(ones1b[:], 1.0)
        W1T = load(cst, 'W1T', A('W1T').transpose([1, 0, 2]))
        P1B = load(cst, 'P1B', A('P1B').rearrange("(a p) o -> p a o", p=P))

        def conv3x3(getsrc, relu, dst):
            with tc.tile_pool(name="cvps", bufs=4, space="PSUM") as cps:
                for mg in range(2):
                    for nh2 in range(2):
                        pt = cps.tile([P, 512], f32, tag="convp")
                        h0 = 16 * nh2
                        k = 0
                        for t in range(9):
                            dy, dx = t // 3, t % 3
                            for kt in range(2):
                                win = getsrc(kt).rearrange("p (h w) -> p h w", h=H + 2)
                                win = win[:, dy + h0:dy + h0 + 16, dx:dx + W]
                                nc.tensor.matmul(pt[:], (W1T[:, t * 2 + kt, mg * P:(mg + 1) * P]),
                                                 (win), start=(k == 0), stop=(k == 17))
                                k += 1
                        fn = FT.Relu if relu else FT.Identity
                        nc.scalar.activation(dst(mg, nh2), pt[:], fn, bias=P1B[:, mg], scale=1.0)

        with tc.tile_pool(name="actA", bufs=1) as actA:
            xh = actA.tile([P, 4, L + DC - 1], bf16, tag="xh")
            SZ = actA.tile([P, 4, L], bf16, tag="SZ")
            comb = actA.tile([P, 4, L], bf16, tag="comb")

            # phase-5 weights prefetched before phase 1 (off the critical path)
            p5w_cm = tc.tile_pool(name="p5w", bufs=1)
            p5w = p5w_cm.__enter__()
            CWT = load(p5w, 'CWT', A('CWT').rearrange("(a p) m -> p a m", p=P))
            CBt = load(p5w, 'CB', A('CB').rearrange("(a p) m -> p a m", p=P))
            XPWT = load(p5w, 'XPWT', A('XPWT').rearrange("b (a p) m -> p b a m", p=P))
            DTWT = load(p5w, 'DTWT', A('DTWT').transpose([1, 0, 2]))
            DTB = load(p5w, 'DTB', A('DTB').rearrange("(a p) m -> p a m", p=P))
            ASC = load(p5w, 'ASC', A('ASC').rearrange("(a p) m -> p a m", p=P))
            DPP = load(p5w, 'DPP', A('DPP').rearrange("(a p) m -> p a m", p=P))
            SELJ = load(p5w, 'SELJ')

            with tc.tile_pool(name="pA", bufs=1) as pA:
                xcn = pA.tile([P, 2, L], bf16, tag="xcn")
                hsT = pA.tile([P, 2, L], bf16, tag="hsT")
                # ===== phase 1+2: conv1 + LN
                with tc.tile_pool(name="p12", bufs=1) as p12:
                    XFPAD = load(p12, 'XFPAD', A('XFPAD').rearrange("(a p) f -> p a f", p=P))
                    LNW = load(p12, 'LNW', A('LNW').rearrange("(a p) o -> p a o", p=P))
                    LNB = load(p12, 'LNB', A('LNB').rearrange("(a p) o -> p a o", p=P))
                    xc = p12.tile([P, 2, L], f32, tag="xc")
                    conv3x3(lambda kt: XFPAD[:, kt], False,
                            lambda mg, nh2: xc[:, mg, NH2[nh2]])
                    with tc.tile_pool(name="lnps", bufs=1, space="PSUM") as lps:
                        xc2 = p12.tile([P, 2, L], f32, tag="xc2")
                        for kt in range(2):
                            nc.scalar.activation(xc2[:, kt], xc[:, kt], FT.Square)
                        s1p = lps.tile([1, L], f32, tag="s1")
                        s2p = lps.tile([1, L], f32, tag="s2")
                        for nh2 in range(2):
                            for kt in range(2):
                                nc.tensor.matmul(s1p[:, NH2[nh2]], (onesk[:]), (xc[:, kt, NH2[nh2]]),
                                                 start=(kt == 0), stop=(kt == 1))
                                nc.tensor.matmul(s2p[:, NH2[nh2]], (onesk[:]), (xc2[:, kt, NH2[nh2]]),
                                                 start=(kt == 0), stop=(kt == 1))
                        s12 = p12.tile([1, 2, L], f32, tag="s12")
                        nc.vector.tensor_copy(s12[:, 0], s1p[:])
                        nc.vector.tensor_copy(s12[:, 1], s2p[:])
                        mrep = lps.tile([P, L], f32, tag="mrep")
                        vrep = lps.tile([P, L], f32, tag="vrep")
                        for nh2 in range(2):
                            nc.tensor.matmul(mrep[:, NH2[nh2]], (mean1[:]), (s12[:, 0, NH2[nh2]]),
                                             start=True, stop=True)
                            nc.tensor.matmul(vrep[:, NH2[nh2]], (mean1[:]), (s12[:, 1, NH2[nh2]]),
                                             start=True, stop=True)
                        mu2 = p12.tile([P, L], f32, tag="mu2")
                        nc.scalar.activation(mu2[:], mrep[:], FT.Square)
                        varr = p12.tile([P, L], f32, tag="varr")
                        nc.vector.tensor_tensor(varr[:], vrep[:], mu2[:], OP.subtract)
                        # 1/sqrt(var+eps) = exp(-0.5*ln(var+eps)): stays in the
                        # exp/ln table set and avoids the 6.5us single-rate
                        # vector reciprocal.
                        stdt = p12.tile([P, L], f32, tag="stdt")
                        nc.scalar.activation(stdt[:], varr[:], FT.Ln, bias=epsb[:])
                        inv = p12.tile([P, L], f32, tag="inv")
                        nc.scalar.activation(inv[:], stdt[:], FT.Exp, scale=-0.5)
                        for kt in range(2):
                            t1 = p12.tile([P, L], f32, tag="lnt1")
                            nc.vector.tensor_tensor(t1[:], xc[:, kt], mrep[:], OP.subtract)
                            t2 = p12.tile([P, L], f32, tag="lnt2")
                            nc.gpsimd.tensor_tensor(t2[:], t1[:], inv[:], OP.mult)
                            nc.scalar.activation(xcn[:, kt], t2[:], FT.Identity,
                                                 bias=LNB[:, kt], scale=LNW[:, kt])

                # ===== phase 3: attention (head split across the core pair;
                # partial o-proj merged with a pairwise AllReduce)
                with tc.tile_pool(name="p3", bufs=2) as p3, \
                     tc.tile_pool(name="ardA", bufs=1, space="DRAM") as ardA:
                    QWT = load(p3, 'QWT', A('QWT').rearrange("h (a p) m -> p h a m", p=P))
                    KWT = load(p3, 'KWT', A('KWT').rearrange("h (a p) m -> p h a m", p=P))
                    VWR = load(p3, 'VWR', A('VWR').rearrange("h (a p) m -> p h a m", p=P))
                    QB = load(p3, 'QB', A('QB').transpose([1, 0, 2]))
                    KB = load(p3, 'KB', A('KB').transpose([1, 0, 2]))
                    VBR = load(p3, 'VBR', A('VBR').transpose([1, 0, 2]))
                    OWT = load(p3, 'OWT', A('OWT').transpose([1, 0, 2]))
                    OB = load(p3, 'OB', A('OB').rearrange("(a p) o -> p a o", p=P))
                    Osb = p3.tile([P, 2, L], bf16, tag="Osb")
                    for h in range(NHL):
                        with tc.tile_pool(name="qkps", bufs=2, space="PSUM") as qps:
                            Qp = qps.tile([DH, L], f32, tag="qkp")
                            Kp = qps.tile([DH, L], f32, tag="qkp")
                            for nh2 in range(2):
                                for kt in range(2):
                                    nc.tensor.matmul(Qp[:, NH2[nh2]], (QWT[:, h, kt]),
                                                     (xcn[:, kt, NH2[nh2]]), start=(kt == 0), stop=(kt == 1))
                                    nc.tensor.matmul(Kp[:, NH2[nh2]], (KWT[:, h, kt]),
                                                     (xcn[:, kt, NH2[nh2]]), start=(kt == 0), stop=(kt == 1))
                            Q = p3.tile([DH, L], bf16, tag="Q")
                            Kt = p3.tile([DH, L], bf16, tag="K")
                            nc.scalar.activation(Q[:], Qp[:], FT.Identity, bias=QB[:, h])
                            nc.scalar.activation(Kt[:], Kp[:], FT.Identity, bias=KB[:, h])
                        Vt = p3.tile([P, 8, DH], bf16, tag="Vt")
                        with tc.tile_pool(name="vps", bufs=2, space="PSUM") as vps:
                            for mgr in range(8):
                                vp = vps.tile([P, DH], f32, tag="vp")
                                for kt in range(2):
                                    nc.tensor.matmul(vp[:], (xcn[:, kt, mgr * P:(mgr + 1) * P]),
                                                     (VWR[:, h, kt]), start=(kt == 0), stop=(kt == 1))
                                nc.vector.tensor_tensor(Vt[:, mgr], vp[:], VBR[:, h], OP.add)
                        expt = p3.tile([P, 8, L], bf16, tag="expt")
                        den = p3.tile([1, 2, L], f32, tag="den")
                        with tc.tile_pool(name="sps", bufs=3, space="PSUM") as spsp, \
                             tc.tile_pool(name="dps", bufs=1, space="PSUM") as dpsp:
                            denp = dpsp.tile([1, L], f32, tag="denp")
                            for nkt in range(8):
                                sp = spsp.tile([P, L], f32, tag="sp")
                                for nh2 in range(2):
                                    nc.tensor.matmul(sp[:, NH2[nh2]], (Kt[:, nkt * P:(nkt + 1) * P]),
                                                     (Q[:, NH2[nh2]]), start=True, stop=True)
                                nc.scalar.activation(expt[:, nkt], sp[:], FT.Exp, scale=SQ)
                                for nh2 in range(2):
                                    nc.tensor.matmul(denp[:, NH2[nh2]], (oneskb[:]),
                                                     (expt[:, nkt, NH2[nh2]]),
                                                     start=(nkt == 0), stop=(nkt == 7))
                            nc.scalar.activation(den[:, 0], denp[:], FT.Ln)
                        nc.scalar.activation(den[:, 1], den[:, 0], FT.Exp, scale=-1.0)
                        with tc.tile_pool(name="pvps", bufs=1, space="PSUM") as pvps:
                            denir_p = pvps.tile([P, L], f32, tag="denir")
                            for nh2 in range(2):
                                nc.tensor.matmul(denir_p[:, NH2[nh2]], (ones1[:]),
                                                 (den[:, 1, NH2[nh2]]), start=True, stop=True)
                            denir = p3.tile([P, L], f32, tag="denirs")
                            nc.vector.tensor_copy(denir[:], denir_p[:])
                            attp = pvps.tile([DH, L], f32, tag="attp")
                            for nkt in range(8):
                                for nh2 in range(2):
                                    nc.tensor.matmul(attp[:, NH2[nh2]], (Vt[:, nkt]),
                                                     (expt[:, nkt, NH2[nh2]]),
                                                     start=(nkt == 0), stop=(nkt == 7))
                            att = p3.tile([DH, L], bf16, tag="att")
                            nc.vector.tensor_tensor(att[:], attp[:], denir[:], OP.mult)
                            Oph = pvps.tile([P, 2, L], f32, tag="oph")
                            for mg in range(2):
                                for nh2 in range(2):
                                    nc.tensor.matmul(Oph[:, mg, NH2[nh2]], (OWT[:, h, mg * P:(mg + 1) * P]),
                                                     (att[:, NH2[nh2]]), start=True, stop=True)
                            for mg in range(2):
                                nc.scalar.activation(Osb[:, mg], Oph[:, mg], FT.Identity, bias=OB[:, mg])
                    aOin = ardA.tile([C, L], bf16, tag="aOin")
                    aOout = ardA.tile([C, L], bf16, tag="aOout")
                    nc.sync.dma_start(aOin[:].rearrange("(a p) l -> p a l", p=P), Osb[:])
                    if use_ar:
                        nc.gpsimd.collective_compute("AllReduce", OP.add, replica_groups=group_all,
                                                     ins=[aOin.opt()], outs=[aOout.opt()])
                        nc.sync.dma_start(Osb[:], aOout[:].rearrange("(a p) l -> p a l", p=P))
                    with tc.tile_pool(name="trps", bufs=4, space="PSUM") as tps:
                        for q in range(4):
                            for mg in range(2):
                                for cg in range(2):
                                    tp = tps.tile([P, P], bf16, tag="trp")
                                    src = Osb[:, mg].rearrange("p (a b) -> p a b", b=4)[:, :, q]
                                    nc.tensor.transpose(tp[:], src[:, cg * P:(cg + 1) * P], identb[:])
                                    nc.vector.tensor_copy(hsT[:, cg, q * 256 + mg * P: q * 256 + (mg + 1) * P], tp[:])

                # ===== phase 4: xz projection
                for dt4 in range(4):
                    nc.gpsimd.memset(xh[:, dt4, 0:DC - 1], 0.0)
                with tc.tile_pool(name="p4", bufs=1) as p4:
                    INWT = load(p4, 'INWT', A('INWT').rearrange("(a p) m -> p a m", p=P))
                    with tc.tile_pool(name="xzps", bufs=4, space="PSUM") as xps:
                        for mg in range(8):
                            pt = xps.tile([P, L], f32, tag="xzp")
                            for nh2 in range(2):
                                for kt in range(2):
                                    nc.tensor.matmul(pt[:, NH2[nh2]], (INWT[:, kt, mg * P:(mg + 1) * P]),
                                                     (hsT[:, kt, NH2[nh2]]), start=(kt == 0), stop=(kt == 1))
                            if mg < 4:
                                nc.vector.tensor_copy(xh[:, mg, DC - 1:], pt[:])
                            else:
                                nc.scalar.activation(SZ[:, mg - 4], pt[:], FT.Silu)

            # ===== phase 5: mamba branches (v2; pA closed -> xcn/hsT freed)
            if True:
                with tc.tile_pool(name="brt", bufs=1) as bp, \
                     tc.tile_pool(name="brtmp", bufs=1) as btmp:
                    xms, xdbls, deltas, dus = {}, {}, {}, {}
                    for br in range(3):
                        xms[br] = bp.tile([P, 4, L], bf16, tag=f"xm{br}", name=f"xm{br}")
                        xdbls[br] = bp.tile([48, L], bf16, tag=f"xdbl{br}", name=f"xdbl{br}")
                        deltas[br] = bp.tile([P, 4, L], bf16, tag=f"delta{br}", name=f"delta{br}")
                        dus[br] = bp.tile([P, 4, L], bf16, tag=f"du{br}", name=f"du{br}")
                    # --- 5a: conv1d + silu for all branches
                    with tc.tile_pool(name="xpadp", bufs=1) as xpp, \
                         tc.tile_pool(name="brps", bufs=4, space="PSUM") as bps:
                        for br in range(3):
                            xm = xms[br]
                            if br == 0:
                                xpadv = xh
                            else:
                                xpadv = xpp.tile([P, 4, L + DC - 1], bf16, tag="xpad")
                                for dt4 in range(4):
                                    nc.gpsimd.memset(xpadv[:, dt4, 0:DC - 1], 0.0)
                                    if br == 1:
                                        nc.vector.tensor_copy(xpadv[:, dt4, DC - 1:], xh[:, dt4, DC - 1:][:, ::-1])
                                    else:
                                        nc.vector.tensor_copy(v_jk(xpadv[:, dt4, DC - 1:]), sliced(xh[:, dt4, DC - 1:]))
                            dg = btmp.tile([P, DC, P], bf16, tag="cdiag")
                            for dt4 in range(4):
                                for j in range(DC):
                                    nc.vector.tensor_scalar_mul(dg[:, j], identb[:], CWT[:, dt4, br * DC + j:br * DC + j + 1])
                                pt = bps.tile([P, L], f32, tag="cvp")
                                for nh2 in range(2):
                                    for j in range(DC):
                                        nc.tensor.matmul(pt[:, NH2[nh2]], (dg[:, j]),
                                                         (xpadv[:, dt4, j + nh2 * 512: j + nh2 * 512 + 512]),
                                                         start=(j == 0), stop=(j == DC - 1))
                                nc.scalar.activation(xm[:, dt4], pt[:], FT.Silu,
                                                     bias=CBt[:, dt4, br:br + 1])
                    # --- 5b: x_dbl + softplus + du for all branches
                    with tc.tile_pool(name="xdpp", bufs=2, space="PSUM") as xdpp, \
                         tc.tile_pool(name="dtpp", bufs=2, space="PSUM") as dtpp:
                        for br in range(3):
                            xm, xdbl, delta, du = xms[br], xdbls[br], deltas[br], dus[br]
                            xdp = xdpp.tile([48, L], f32, tag="xdp")
                            for nh2 in range(2):
                                for kt in range(4):
                                    nc.tensor.matmul(xdp[:, NH2[nh2]], (XPWT[:, br, kt]),
                                                     (xm[:, kt, NH2[nh2]]), start=(kt == 0), stop=(kt == 3))
                            nc.vector.tensor_copy(xdbl[:], xdp[:])
                            for dt4 in range(4):
                                dtp = dtpp.tile([P, L], f32, tag="dtp")
                                for nh2 in range(2):
                                    nc.tensor.matmul(dtp[:, NH2[nh2]], (DTWT[:, br, dt4 * P:(dt4 + 1) * P]),
                                                     (xdbl[:DTR, NH2[nh2]]), start=True, stop=True)
                                spe = btmp.tile([P, L], bf16, tag="yf")
                                nc.scalar.activation(spe[:], dtp[:], FT.Exp,
                                                     bias=DTB[:, dt4, br:br + 1])
                                nc.scalar.activation(delta[:, dt4], spe[:], FT.Ln, bias=1.0)
                            nc.vector.tensor_tensor(
                                du[:].rearrange("p a b -> p (a b)"),
                                delta[:].rearrange("p a b -> p (a b)"),
                                xm[:].rearrange("p a b -> p (a b)"), OP.mult)
                    # --- 5c v3: JIT PSUM broadcasts + 4-state chained scans.
                    # The scan chains 4 states in one instruction; zeroing the
                    # first dA column of every state segment makes the chaining
                    # exact (s_0 = dBu_0 regardless of carried state).
                    with tc.tile_pool(name="bcsb", bufs=2) as bcp, \
                         tc.tile_pool(name="sctA", bufs=2) as scpA, \
                         tc.tile_pool(name="sctB", bufs=2) as scpB, \
                         tc.tile_pool(name="sctS", bufs=2) as scpS, \
                         tc.tile_pool(name="bcps", bufs=2, space="PSUM") as bcps, \
                         tc.tile_pool(name="ysps", bufs=2, space="PSUM") as ysps:
                        # pre-zero column 0 of both dA4 buffers once; the exp
                        # activations write only [:, 1:], so the zeros persist
                        # and make the 4-state scan chaining exact.
                        for _ in range(2):
                            t0_ = scpA.tile([P, 4, L], bf16, tag="dA4")
                            nc.vector.memset(t0_[:, :, 0:1], 0.0)
                        for br in range(3):
                            xm, xdbl, delta, du = xms[br], xdbls[br], deltas[br], dus[br]
                            # prebuild B/C broadcast tiles per state-half (shared
                            # across the dt4 loop; bufs=2 pipelines half1 builds
                            # under half0 scans)
                            bch = {}
                            for half in range(nspl // 4):
                                j0 = half * 4
                                bc4 = bcp.tile([P, 4, 2, L], bf16, tag="bc4")
                                bch[half] = bc4
                                for jj in range(4):
                                    j = j0 + jj
                                    brp = bcps.tile([P, L], f32, tag="bcj")
                                    for nh2 in range(2):
                                        nc.tensor.matmul(brp[:, NH2[nh2]], (SELJ[:, j * P:(j + 1) * P]),
                                                         (xdbl[:, NH2[nh2]]), start=True, stop=True)
                                    nc.scalar.copy(bc4[:, jj, 0], brp[:])
                                    crp = bcps.tile([P, L], f32, tag="bcj")
                                    for nh2 in range(2):
                                        nc.tensor.matmul(crp[:, NH2[nh2]], (SELJ[:, (nspl + j) * P:(nspl + j + 1) * P]),
                                                         (xdbl[:, NH2[nh2]]), start=True, stop=True)
                                    nc.scalar.copy(bc4[:, jj, 1], crp[:])
                            # Software-pipelined: each half's ypk/yacc (and each
                            # dt4's 5d) is emitted AFTER the next half's gpsimd
                            # dBu, so the scan never waits behind the previous
                            # ypk in the gpsimd queue.
                            def mk_ypk(dBu4_, sout_, bc4_, j0_, yaccp_):
                                def emit():
                                    nc.gpsimd.tensor_tensor(
                                        dBu4_[:], sout_[:], bc4_[:, :, 1], OP.mult)
                                    for jj in range(4):
                                        j = j0_ + jj
                                        for nh2 in range(2):
                                            nc.tensor.matmul(yaccp_[:, NH2[nh2]], (identb[:]),
                                                             (dBu4_[:, jj, NH2[nh2]]),
                                                             start=(j == 0), stop=(j == nspl - 1))
                                return emit

                            def mk_5d(dt4_, yaccp_, br_, xm_):
                                def emit():
                                    yf = btmp.tile([P, L], bf16, tag="yf")
                                    nc.vector.scalar_tensor_tensor(yf[:], xm_[:, dt4_], DPP[:, dt4_, br_:br_ + 1],
                                                                   yaccp_[:], OP.mult, OP.add)
                                    if br_ == 0:
                                        nc.vector.tensor_tensor(comb[:, dt4_], yf[:], SZ[:, dt4_], OP.mult)
                                    elif br_ == 1:
                                        yg = btmp.tile([P, L], bf16, tag="yg")
                                        nc.gpsimd.tensor_tensor(yg[:], yf[:], SZ[:, dt4_][:, ::-1], OP.mult)
                                        nc.vector.tensor_tensor(comb[:, dt4_], comb[:, dt4_], yg[:][:, ::-1], OP.add)
                                    else:
                                        yg = btmp.tile([P, L], bf16, tag="yg")
                                        nc.gpsimd.tensor_tensor(v_jk(yg[:]), v_jk(yf[:]), sliced(SZ[:, dt4_]), OP.mult)
                                        nc.vector.tensor_tensor(comb[:, dt4_], comb[:, dt4_], unsliced(yg[:]), OP.add)
                                return emit

                            pend = []
                            for dt4 in range(4):
                                yaccp = ysps.tile([P, L], f32, tag="yacc")
                                for half in range(nspl // 4):
                                    j0 = half * 4
                                    bc4 = bch[half]
                                    dA4 = scpA.tile([P, 4, L], bf16, tag="dA4")
                                    for jj in range(4):
                                        j = j0 + jj
                                        nc.scalar.activation(dA4[:, jj, 1:], delta[:, dt4, 1:], FT.Exp,
                                                             scale=ASC[:, dt4, br * nspl + j:br * nspl + j + 1])
                                    dBu4 = scpB.tile([P, 4, L], bf16, tag="dBu4")
                                    for jj in range(4):
                                        eng = nc.gpsimd if jj == 3 else nc.vector
                                        eng.tensor_tensor(dBu4[:, jj], du[:, dt4], bc4[:, jj, 0], OP.mult)
                                    for f in pend:
                                        f()
                                    pend = []
                                    sout = scpS.tile([P, 4, L], bf16, tag="sout")
                                    nc.vector.tensor_tensor_scan(
                                        sout[:].rearrange("p a b -> p (a b)"),
                                        dA4[:].rearrange("p a b -> p (a b)"),
                                        dBu4[:].rearrange("p a b -> p (a b)"),
                                        0.0, OP.mult, OP.add)
                                    pend.append(mk_ypk(dBu4, sout, bc4, j0, yaccp))
                                pend.append(mk_5d(dt4, yaccp, br, xm))
                            for f in pend:
                                f()

            p5w_cm.__exit__(None, None, None)
            # ===== phases 6+7 share a tail pool (p7 weights prefetch first)
            with tc.tile_pool(name="p7", bufs=1) as p7:
                Mfull = p7.tile([P, 2, L], bf16, tag="Mfull")
                P2T = load(p7, 'P2T', A('P2T').rearrange("(a p) m -> p a m", p=P))
                P2B = load(p7, 'P2B', A('P2B').rearrange("(a p) o -> p a o", p=P))
                F1T = load(p7, 'F1T', A('F1T').rearrange("(a p) m -> p a m", p=P))
                F1B = load(p7, 'F1B')
                DWC = load(p7, 'DWC')
                DWB = load(p7, 'DWB')
                XSKIP = load(p7, 'XSKIP')
                # ===== phase 6: out projection + AllReduce
                with tc.tile_pool(name="p6", bufs=1) as p6, \
                     tc.tile_pool(name="mps", bufs=2, space="PSUM") as mps, \
                     tc.tile_pool(name="ardram", bufs=1, space="DRAM") as ard:
                    OUTWT = load(p6, 'OUTWT', A('OUTWT').rearrange("(a p) m -> p a m", p=P))
                    Mpart = p6.tile([P, 2, L], bf16, tag="mpart")
                    for mg in range(2):
                        mp = mps.tile([P, L], f32, tag="mp")
                        for nh2 in range(2):
                            for kt in range(4):
                                nc.tensor.matmul(mp[:, NH2[nh2]], (OUTWT[:, kt, mg * P:(mg + 1) * P]),
                                                 (comb[:, kt, NH2[nh2]]), start=(kt == 0), stop=(kt == 3))
                        nc.scalar.copy(Mpart[:, mg], mp[:])
                    bin_ = ard.tile([C, L], bf16, tag="arin")
                    bout = ard.tile([C, L], bf16, tag="arout")
                    nc.sync.dma_start(bin_[:].rearrange("(a p) l -> p a l", p=P), Mpart[:])
                    if use_ar:
                        nc.gpsimd.collective_compute("AllReduce", OP.add, replica_groups=group_all,
                                                     ins=[bin_.opt()], outs=[bout.opt()])
                        nc.sync.dma_start(Mfull[:], bout[:].rearrange("(a p) l -> p a l", p=P))
                    else:
                        nc.sync.dma_start(Mfull[:], bin_[:].rearrange("(a p) l -> p a l", p=P))

                # ===== phase 7: conv1#2, conv2, fc1, dw + residual
                mpad = p7.tile([P, 2, HP], bf16, tag="mpad")
                for mg in range(2):
                    nc.gpsimd.memset(mpad[:, mg], 0.0)
                    dst = mpad[:, mg].rearrange("p (h w) -> p h w", h=H + 2)[:, 1:H + 1, 1:W + 1]
                    nc.vector.tensor_copy(dst, Mfull[:, mg].rearrange("p (h w) -> p h w", h=H))
                c1 = p7.tile([P, 2, L], bf16, tag="c1")
                conv3x3(lambda kt: mpad[:, kt], True,
                        lambda mg, nh2: c1[:, mg, NH2[nh2]])
                c2 = p7.tile([P, 2, L], bf16, tag="c2")
                with tc.tile_pool(name="c2ps", bufs=2, space="PSUM") as cps:
                    for mg in range(2):
                        for nh2 in range(2):
                            pt = cps.tile([P, 512], f32, tag="c2p")
                            for kt in range(2):
                                nc.tensor.matmul(pt[:], (P2T[:, kt, mg * P:(mg + 1) * P]),
                                                 (c1[:, kt, NH2[nh2]]), start=(kt == 0), stop=(kt == 1))
                            nc.scalar.activation(c2[:, mg, NH2[nh2]], pt[:], FT.Relu, bias=P2B[:, mg])
                    xfpad = p7.tile([P, HP], bf16, tag="xfpad")
                    nc.gpsimd.memset(xfpad[:], 0.0)
                    for nh2 in range(2):
                        pt = cps.tile([P, 512], f32, tag="fcp")
                        for kt in range(2):
                            nc.tensor.matmul(pt[:], (F1T[:, kt]), (c2[:, kt, NH2[nh2]]),
                                             start=(kt == 0), stop=(kt == 1))
                        dstv = xfpad[:].rearrange("p (h w) -> p h w", h=H + 2)[:, 1 + 16 * nh2:17 + 16 * nh2, 1:W + 1]
                        nc.scalar.activation(dstv, pt[:].rearrange("p (h w) -> p h w", h=16),
                                             FT.Identity, bias=F1B[:])
                    dwg = p7.tile([P, 9, P], bf16, tag="dwg")
                    for t in range(9):
                        nc.vector.tensor_scalar_mul(dwg[:, t], identb[:], DWC[:, t:t + 1])
                    outsb = p7.tile([P, L], f32, tag="outsb")
                    for nh2 in range(2):
                        pt = cps.tile([P, 512], f32, tag="dwp")
                        h0 = 16 * nh2
                        for t in range(9):
                            dy, dx = t // 3, t % 3
                            win = xfpad[:].rearrange("p (h w) -> p h w", h=H + 2)
                            win = win[:, dy + h0:dy + h0 + 16, dx:dx + W]
                            nc.tensor.matmul(pt[:], (dwg[:, t]), (win), start=(t == 0), stop=(t == 8))
                        dwt = p7.tile([P, 512], f32, tag="dwt")
                        nc.scalar.activation(dwt[:], pt[:], FT.Identity, bias=DWB[:])
                        nc.vector.tensor_tensor(outsb[:, NH2[nh2]], dwt[:],
                                                XSKIP[:, NH2[nh2]], OP.add)
                    nc.sync.dma_start(OUTT.ap(), outsb[:])


_CACHE = {}


def _build():
    if 'nc' in _CACHE:
        return
    from concourse import bacc
    nc = bacc.Bacc(target_bir_lowering=False)
    group = [[0, 1], [2, 3], [4, 5], [6, 7]]
    build(nc, use_ar=True, group_all=group, nspl=8)
    nc.compile()
    _CACHE['nc'] = nc


def kernel(**inputs):
    _build()
    from concourse.bass_utils import run_bass_kernel_spmd
    nc = _CACHE['nc']
    in_maps = [host_prep(inputs, core, nspl=8) for core in range(8)]
    res = run_bass_kernel_spmd(nc, in_maps, core_ids=list(range(8)))
    out = np.zeros((B, C, H * W), np.float32)
    for core in range(8):
        b, s = core // 2, core % 2
        out[b, s * 128:(s + 1) * 128] = res.results[core]['OUT']
    return out.reshape(B, C, H, W)



# revision 57
# speedup vs baseline: 1.0561x; 1.0519x over previous
"""Self-contained Trainium2 Bass kernel for nn_Att_MambaLayer_12034498363969.

kernel(**inputs) takes FULL unsharded inputs, returns the FULL output.
Sharding: 8 NeuronCores = 4 batches x 2. The two cores of a batch pair
duplicate the PE-heavy phases (conv1, layernorm, attention, xz projection,
mamba branch prelims) and split the dominant vector-engine work -- the 16
selective-scan states per mamba branch -- 50/50. A single pairwise
AllReduce on the out-projection partial merges the halves; final convs are
duplicated and fc1/depthwise-conv are split by output channel half,
assembled on host.

v2 phase-5 design (calibrated on hw):
 - dA_n = exp(-n * delta) via one scalar-engine EXP per (br, dt4, state)
   (A = -n exactly for this model).
 - B/C state broadcasts via PE sel-matmuls, copied PSUM->SBUF on scalar.
 - dBu multiply + scans on Vector (scan is Vector-only; ~2.2ns/elem).
 - y = sout*C multiplies on GpSimd (keeps Vector for scans).
 - y state-sum via PE identity-matmul accumulation in PSUM (replaces the
   baseline's 84 gpsimd-issued accum-DMAs).
 - 5d gating reads the PSUM y-sum directly via Vector STT; branch combine
   via plain TT adds (reversed / unsliced views), no PSUM round-trip.
"""
import sys
sys.path.insert(0, '/opt/trn_rl_repo')
import numpy as np

import concourse.bass as bass
import concourse.mybir as mybir
import concourse.tile as tile
from concourse.masks import make_identity

f32 = mybir.dt.float32
bf16 = mybir.dt.bfloat16
FT = mybir.ActivationFunctionType
OP = mybir.AluOpType

B, C, H, W = 4, 256, 32, 32
L = H * W
DS, DC, NSL, NH, DH = 16, 4, 16, 2, 128
DI, DTR = 512, 16
P = 128
HP = (H + 2) * (W + 2)
SQ = 1.0 / float(np.sqrt(DH))


BF16_IN = {'W1T', 'QWT', 'KWT', 'VWR', 'OWT', 'INWT', 'XPWT', 'DTWT',
           'OUTWT', 'P2T', 'F1T', 'SELB', 'SELC', 'XFPAD'}


def host_prep(inp, core, nspl=8):
    import ml_dtypes
    b, s = core // 2, core % 2
    g = lambda k: np.asarray(inp[k], np.float32)
    x = g('x')
    d = {}
    x_flat = np.transpose(x, (0, 2, 1, 3)).reshape(B, C, H, W)[b]
    xfp = np.zeros((C, H + 2, W + 2), np.float32)
    xfp[:, 1:-1, 1:-1] = x_flat
    d['XFPAD'] = xfp.reshape(C, HP)
    d['XSKIP'] = x[b].reshape(C, L)[s * P:(s + 1) * P].copy()
    w1 = g('proj1_w')
    w1t = np.zeros((18, P, C), np.float32)
    for t in range(9):
        dy, dx = t // 3, t % 3
        for kt in range(2):
            w1t[t * 2 + kt] = w1[:, kt * P:(kt + 1) * P, dy, dx].T
    d['W1T'] = w1t
    d['P1B'] = g('proj1_b').reshape(C, 1)
    d['LNW'] = g('norm_w').reshape(C, 1)
    d['LNB'] = g('norm_b').reshape(C, 1)
    qw, kw, vw = g('q_w'), g('k_w'), g('v_w')
    hs_ = [s]  # this core's attention head; partial o-proj merged by AllReduce
    d['QWT'] = np.stack([qw[h * DH:(h + 1) * DH].T for h in hs_])
    d['KWT'] = np.stack([kw[h * DH:(h + 1) * DH].T for h in hs_])
    d['VWR'] = np.stack([vw[h * DH:(h + 1) * DH].T for h in hs_])
    d['QB'] = np.stack([g('q_b')[h * DH:(h + 1) * DH].reshape(DH, 1) for h in hs_])
    d['KB'] = np.stack([g('k_b')[h * DH:(h + 1) * DH].reshape(DH, 1) for h in hs_])
    d['VBR'] = np.stack([np.tile(g('v_b')[h * DH:(h + 1) * DH][None, :], (P, 1)) for h in hs_])
    d['OWT'] = np.stack([g('o_w')[:, h * DH:(h + 1) * DH].T for h in hs_])
    d['OB'] = g('o_b').reshape(C, 1) if s == 0 else np.zeros((C, 1), np.float32)
    d['INWT'] = g('in_w').T.copy()
    cwn, cbn = ['cw', 'cbw', 'csw'], ['cb', 'cbb', 'csb']
    xpn, dwn, dbn = ['xpw', 'xpbw', 'xpsw'], ['dtw', 'dtbw', 'dtsw'], ['dtb', 'dtbb', 'dtsb']
    aln, ddn = ['Alog', 'Ablog', 'Aslog'], ['D', 'Db', 'Ds']
    d['CWT'] = np.concatenate([g(cwn[i])[:, 0, :] for i in range(3)], axis=1)  # [512,12]
    d['CB'] = np.stack([g(cbn[i]) for i in range(3)], 1)
    ns0 = np.arange(s * nspl, (s + 1) * nspl) if nspl < DS else np.arange(DS)
    xps = []
    for i in range(3):
        xp = g(xpn[i]).T  # [DI, DTR+2*DS]
        xr = np.zeros_like(xp)
        xr[:, :DTR] = xp[:, :DTR]
        for j, n in enumerate(ns0):
            xr[:, DTR + j] = xp[:, DTR + n]
            xr[:, DTR + DS + j] = xp[:, DTR + DS + n]
        xps.append(xr)
    d['XPWT'] = np.stack(xps)
    d['DTWT'] = np.stack([g(dwn[i]).T for i in range(3)])
    d['DTB'] = np.stack([g(dbn[i]) for i in range(3)], 1)
    ns = np.arange(s * nspl, (s + 1) * nspl) if nspl < DS else np.arange(DS)
    asc = np.zeros((DI, 3 * nspl), np.float32)
    for i in range(3):
        A = -np.exp(g(aln[i]))
        for j, n in enumerate(ns):
            asc[:, i * nspl + j] = A[:, n]
    d['ASC'] = asc
    selb = np.zeros((3, nspl, 48, P), np.float32)
    selc = np.zeros((3, nspl, 48, P), np.float32)
    for i in range(3):
        for j, n in enumerate(ns):
            selb[i, j, DTR + n, :] = 1.0
            selc[i, j, DTR + DS + n, :] = 1.0
    d['SELB'] = selb
    d['SELC'] = selc
    # one-hot selectors for the reordered xdbl layout (state j's B at row
    # DTR+j, C at DTR+DS+j) -- identical for every branch and core.
    selj = np.zeros((48, 2 * nspl * P), np.float32)
    for j in range(nspl):
        selj[DTR + j, j * P:(j + 1) * P] = 1.0
        selj[DTR + DS + j, (nspl + j) * P:(nspl + j + 1) * P] = 1.0
    d['SELJ'] = selj
    dpp = np.stack([g(ddn[i]) for i in range(3)], 1)
    d['DPP'] = dpp if (s == 0 or nspl == DS) else np.zeros_like(dpp)
    d['OUTWT'] = g('outw').T.copy()
    d['P2T'] = g('proj2_w')[:, :, 0, 0].T.copy()
    d['P2B'] = g('proj2_b').reshape(C, 1)
    own = slice(s * P, (s + 1) * P)
    d['F1T'] = g('fc1_w')[own].T.copy()
    d['F1B'] = g('fc1_b')[own].reshape(P, 1)
    d['DWC'] = g('dw_w')[:, 0][own].reshape(P, 9)
    d['DWB'] = g('dw_b')[own].reshape(P, 1)
    for k in BF16_IN | {'SELJ'}:
        d[k] = d[k].astype(ml_dtypes.bfloat16)
    return d


NHL = 1  # heads computed per core (head split across the pair)

IN_SHAPES = [
    ('XFPAD', (C, HP)), ('XSKIP', (P, L)), ('W1T', (18, P, C)), ('P1B', (C, 1)),
    ('LNW', (C, 1)), ('LNB', (C, 1)),
    ('QWT', (NHL, C, DH)), ('KWT', (NHL, C, DH)), ('VWR', (NHL, C, DH)),
    ('QB', (NHL, DH, 1)), ('KB', (NHL, DH, 1)), ('VBR', (NHL, P, DH)),
    ('OWT', (NHL, DH, C)), ('OB', (C, 1)), ('INWT', (C, 2 * DI)),
    ('CWT', (DI, 12)), ('CB', (DI, 3)), ('XPWT', (3, DI, 48)),
    ('DTWT', (3, DTR, DI)), ('DTB', (DI, 3)),
    ('DPP', (DI, 3)), ('OUTWT', (DI, C)), ('P2T', (C, C)), ('P2B', (C, 1)),
    ('F1T', (C, P)), ('F1B', (P, 1)), ('DWC', (P, 9)), ('DWB', (P, 1)),
]


def build(nc, use_ar, group_all, nspl=8):
    din = {}
    for name, shape in IN_SHAPES + [('ASC', (DI, 3 * nspl)),
                                    ('SELB', (3, nspl, 48, P)),
                                    ('SELC', (3, nspl, 48, P)),
                                    ('SELJ', (48, 2 * nspl * P))]:
        dt_ = bf16 if (name in BF16_IN or name == 'SELJ') else f32
        din[name] = nc.dram_tensor(name, list(shape), dt_, kind="ExternalInput")
    OUTT = nc.dram_tensor('OUT', [P, L], f32, kind="ExternalOutput")
    with tile.TileContext(nc) as tc:
        prog(tc, din, OUTT, use_ar, group_all, nspl)
    return din, OUTT


def prog(tc, din, OUTT, use_ar, group_all, nspl):
    nc = tc.nc
    vengs = [nc.vector, nc.gpsimd]
    ectr = [0]

    def ve():
        ectr[0] += 1
        return vengs[ectr[0] % 2]

    A = lambda n: din[n].ap()
    NH2 = (slice(0, 512), slice(512, 1024))
    JJ = L // NSL

    def load(pool, name, view=None, tag=None):
        src = view if view is not None else A(name)
        t = pool.tile(list(src.shape), src.dtype, tag=tag or name)
        nc.sync.dma_start(t[:], src)
        return t

    def sliced(t2d):
        return t2d.rearrange("p (k j) -> p j k", k=NSL)

    def v_jk(t2d):
        return t2d.rearrange("p (j k) -> p j k", j=JJ)

    def unsliced(t2d):
        return t2d.rearrange("p (j k) -> p k j", j=JJ)

    with tc.tile_pool(name="cst", bufs=1) as cst:
        ident = cst.tile([P, P], f32, tag="ident")
        make_identity(nc, ident[:])
        identb = cst.tile([P, P], bf16, tag="identb")
        nc.scalar.copy(identb[:], ident[:])
        ones1 = cst.tile([1, P], f32, tag="ones1")
        nc.gpsimd.memset(ones1[:], 1.0)
        mean1 = cst.tile([1, P], f32, tag="mean1")
        nc.gpsimd.memset(mean1[:], 1.0 / C)
        onesk = cst.tile([P, 1], f32, tag="onesk")
        nc.gpsimd.memset(onesk[:], 1.0)
        epsb = cst.tile([P, 1], f32, tag="epsb")
        nc.gpsimd.memset(epsb[:], 1e-5)
        oneskb = cst.tile([P, 1], bf16, tag="oneskb")
        nc.gpsimd.memset(oneskb[:], 1.0)
        ones1b = cst.tile([1, P], bf16, tag="ones1b")
        nc.gpsimd.memset(ones1b[:], 1.0)
        W1T = load(cst, 'W1T', A('W1T').transpose([1, 0, 2]))
        P1B = load(cst, 'P1B', A('P1B').rearrange("(a p) o -> p a o", p=P))

        def conv3x3(getsrc, relu, dst):
            with tc.tile_pool(name="cvps", bufs=4, space="PSUM") as cps:
                for mg in range(2):
                    for nh2 in range(2):
                        pt = cps.tile([P, 512], f32, tag="convp")
                        h0 = 16 * nh2
                        k = 0
                        for t in range(9):
                            dy, dx = t // 3, t % 3
                            for kt in range(2):
                                win = getsrc(kt).rearrange("p (h w) -> p h w", h=H + 2)
                                win = win[:, dy + h0:dy + h0 + 16, dx:dx + W]
                                nc.tensor.matmul(pt[:], (W1T[:, t * 2 + kt, mg * P:(mg + 1) * P]),
                                                 (win), start=(k == 0), stop=(k == 17))
                                k += 1
                        fn = FT.Relu if relu else FT.Identity
                        nc.scalar.activation(dst(mg, nh2), pt[:], fn, bias=P1B[:, mg], scale=1.0)

        with tc.tile_pool(name="actA", bufs=1) as actA:
            xh = actA.tile([P, 4, L + DC - 1], bf16, tag="xh")
            SZ = actA.tile([P, 4, L], bf16, tag="SZ")
            comb = actA.tile([P, 4, L], bf16, tag="comb")

            # phase-5 weights prefetched before phase 1 (off the critical path)
            p5w_cm = tc.tile_pool(name="p5w", bufs=1)
            p5w = p5w_cm.__enter__()
            CWT = load(p5w, 'CWT', A('CWT').rearrange("(a p) m -> p a m", p=P))
            CBt = load(p5w, 'CB', A('CB').rearrange("(a p) m -> p a m", p=P))
            XPWT = load(p5w, 'XPWT', A('XPWT').rearrange("b (a p) m -> p b a m", p=P))
            DTWT = load(p5w, 'DTWT', A('DTWT').transpose([1, 0, 2]))
            DTB = load(p5w, 'DTB', A('DTB').rearrange("(a p) m -> p a m", p=P))
            ASC = load(p5w, 'ASC', A('ASC').rearrange("(a p) m -> p a m", p=P))
            DPP = load(p5w, 'DPP', A('DPP').rearrange("(a p) m -> p a m", p=P))
            SELJ = load(p5w, 'SELJ')

            with tc.tile_pool(name="pA", bufs=1) as pA:
                xcn = pA.tile([P, 2, L], bf16, tag="xcn")
                hsT = pA.tile([P, 2, L], bf16, tag="hsT")
                # ===== phase 1+2: conv1 + LN
                with tc.tile_pool(name="p12", bufs=1) as p12:
                    XFPAD = load(p12, 'XFPAD', A('XFPAD').rearrange("(a p) f -> p a f", p=P))
                    LNW = load(p12, 'LNW', A('LNW').rearrange("(a p) o -> p a o", p=P))
                    LNB = load(p12, 'LNB', A('LNB').rearrange("(a p) o -> p a o", p=P))
                    xc = p12.tile([P, 2, L], f32, tag="xc")
                    conv3x3(lambda kt: XFPAD[:, kt], False,
                            lambda mg, nh2: xc[:, mg, NH2[nh2]])
                    with tc.tile_pool(name="lnps", bufs=1, space="PSUM") as lps:
                        xc2 = p12.tile([P, 2, L], f32, tag="xc2")
                        for kt in range(2):
                            nc.scalar.activation(xc2[:, kt], xc[:, kt], FT.Square)
                        s1p = lps.tile([1, L], f32, tag="s1")
                        s2p = lps.tile([1, L], f32, tag="s2")
                        for nh2 in range(2):
                            for kt in range(2):
                                nc.tensor.matmul(s1p[:, NH2[nh2]], (onesk[:]), (xc[:, kt, NH2[nh2]]),
                                                 start=(kt == 0), stop=(kt == 1))
                                nc.tensor.matmul(s2p[:, NH2[nh2]], (onesk[:]), (xc2[:, kt, NH2[nh2]]),
                                                 start=(kt == 0), stop=(kt == 1))
                        s12 = p12.tile([1, 2, L], f32, tag="s12")
                        nc.vector.tensor_copy(s12[:, 0], s1p[:])
                        nc.vector.tensor_copy(s12[:, 1], s2p[:])
                        mrep = lps.tile([P, L], f32, tag="mrep")
                        vrep = lps.tile([P, L], f32, tag="vrep")
                        for nh2 in range(2):
                            nc.tensor.matmul(mrep[:, NH2[nh2]], (mean1[:]), (s12[:, 0, NH2[nh2]]),
                                             start=True, stop=True)
                            nc.tensor.matmul(vrep[:, NH2[nh2]], (mean1[:]), (s12[:, 1, NH2[nh2]]),
                                             start=True, stop=True)
                        mu2 = p12.tile([P, L], f32, tag="mu2")
                        nc.scalar.activation(mu2[:], mrep[:], FT.Square)
                        varr = p12.tile([P, L], f32, tag="varr")
                        nc.vector.tensor_tensor(varr[:], vrep[:], mu2[:], OP.subtract)
                        # 1/sqrt(var+eps) = exp(-0.5*ln(var+eps)): stays in the
                        # exp/ln table set and avoids the 6.5us single-rate
                        # vector reciprocal.
                        stdt = p12.tile([P, L], f32, tag="stdt")
                        nc.scalar.activation(stdt[:], varr[:], FT.Ln, bias=epsb[:])
                        inv = p12.tile([P, L], f32, tag="inv")
                        nc.scalar.activation(inv[:], stdt[:], FT.Exp, scale=-0.5)
                        for kt in range(2):
                            t1 = p12.tile([P, L], f32, tag="lnt1")
                            nc.vector.tensor_tensor(t1[:], xc[:, kt], mrep[:], OP.subtract)
                            t2 = p12.tile([P, L], f32, tag="lnt2")
                            nc.gpsimd.tensor_tensor(t2[:], t1[:], inv[:], OP.mult)
                            nc.scalar.activation(xcn[:, kt], t2[:], FT.Identity,
                                                 bias=LNB[:, kt], scale=LNW[:, kt])

                # ===== phase 3: attention (head split across the core pair;
                # partial o-proj merged with a pairwise AllReduce)
                with tc.tile_pool(name="p3", bufs=2) as p3, \
                     tc.tile_pool(name="ardA", bufs=1, space="DRAM") as ardA:
                    QWT = load(p3, 'QWT', A('QWT').rearrange("h (a p) m -> p h a m", p=P))
                    KWT = load(p3, 'KWT', A('KWT').rearrange("h (a p) m -> p h a m", p=P))
                    VWR = load(p3, 'VWR', A('VWR').rearrange("h (a p) m -> p h a m", p=P))
                    QB = load(p3, 'QB', A('QB').transpose([1, 0, 2]))
                    KB = load(p3, 'KB', A('KB').transpose([1, 0, 2]))
                    VBR = load(p3, 'VBR', A('VBR').transpose([1, 0, 2]))
                    OWT = load(p3, 'OWT', A('OWT').transpose([1, 0, 2]))
                    OB = load(p3, 'OB', A('OB').rearrange("(a p) o -> p a o", p=P))
                    Osb = p3.tile([P, 2, L], bf16, tag="Osb")
                    for h in range(NHL):
                        with tc.tile_pool(name="qkps", bufs=2, space="PSUM") as qps:
                            Qp = qps.tile([DH, L], f32, tag="qkp")
                            Kp = qps.tile([DH, L], f32, tag="qkp")
                            for nh2 in range(2):
                                for kt in range(2):
                                    nc.tensor.matmul(Qp[:, NH2[nh2]], (QWT[:, h, kt]),
                                                     (xcn[:, kt, NH2[nh2]]), start=(kt == 0), stop=(kt == 1))
                                    nc.tensor.matmul(Kp[:, NH2[nh2]], (KWT[:, h, kt]),
                                                     (xcn[:, kt, NH2[nh2]]), start=(kt == 0), stop=(kt == 1))
                            Q = p3.tile([DH, L], bf16, tag="Q")
                            Kt = p3.tile([DH, L], bf16, tag="K")
                            nc.scalar.activation(Q[:], Qp[:], FT.Identity, bias=QB[:, h])
                            nc.scalar.activation(Kt[:], Kp[:], FT.Identity, bias=KB[:, h])
                        Vt = p3.tile([P, 8, DH], bf16, tag="Vt")
                        with tc.tile_pool(name="vps", bufs=2, space="PSUM") as vps:
                            for mgr in range(8):
                                vp = vps.tile([P, DH], f32, tag="vp")
                                for kt in range(2):
                                    nc.tensor.matmul(vp[:], (xcn[:, kt, mgr * P:(mgr + 1) * P]),
                                                     (VWR[:, h, kt]), start=(kt == 0), stop=(kt == 1))
                                nc.vector.tensor_tensor(Vt[:, mgr], vp[:], VBR[:, h], OP.add)
                        expt = p3.tile([P, 8, L], bf16, tag="expt")
                        den = p3.tile([1, 2, L], f32, tag="den")
                        with tc.tile_pool(name="sps", bufs=3, space="PSUM") as spsp, \
                             tc.tile_pool(name="dps", bufs=1, space="PSUM") as dpsp:
                            denp = dpsp.tile([1, L], f32, tag="denp")
                            for nkt in range(8):
                                sp = spsp.tile([P, L], f32, tag="sp")
                                for nh2 in range(2):
                                    nc.tensor.matmul(sp[:, NH2[nh2]], (Kt[:, nkt * P:(nkt + 1) * P]),
                                                     (Q[:, NH2[nh2]]), start=True, stop=True)
                                nc.scalar.activation(expt[:, nkt], sp[:], FT.Exp, scale=SQ)
                                for nh2 in range(2):
                                    nc.tensor.matmul(denp[:, NH2[nh2]], (oneskb[:]),
                                                     (expt[:, nkt, NH2[nh2]]),
                                                     start=(nkt == 0), stop=(nkt == 7))
                            nc.scalar.activation(den[:, 0], denp[:], FT.Ln)
                        nc.scalar.activation(den[:, 1], den[:, 0], FT.Exp, scale=-1.0)
                        with tc.tile_pool(name="pvps", bufs=1, space="PSUM") as pvps:
                            denir_p = pvps.tile([P, L], f32, tag="denir")
                            for nh2 in range(2):
                                nc.tensor.matmul(denir_p[:, NH2[nh2]], (ones1[:]),
                                                 (den[:, 1, NH2[nh2]]), start=True, stop=True)
                            denir = p3.tile([P, L], f32, tag="denirs")
                            nc.vector.tensor_copy(denir[:], denir_p[:])
                            attp = pvps.tile([DH, L], f32, tag="attp")
                            for nkt in range(8):
                                for nh2 in range(2):
                                    nc.tensor.matmul(attp[:, NH2[nh2]], (Vt[:, nkt]),
                                                     (expt[:, nkt, NH2[nh2]]),
                                                     start=(nkt == 0), stop=(nkt == 7))
                            att = p3.tile([DH, L], bf16, tag="att")
                            nc.vector.tensor_tensor(att[:], attp[:], denir[:], OP.mult)
                            Oph = pvps.tile([P, 2, L], f32, tag="oph")
                            for mg in range(2):
                                for nh2 in range(2):
                                    nc.tensor.matmul(Oph[:, mg, NH2[nh2]], (OWT[:, h, mg * P:(mg + 1) * P]),
                                                     (att[:, NH2[nh2]]), start=True, stop=True)
                            for mg in range(2):
                                nc.scalar.activation(Osb[:, mg], Oph[:, mg], FT.Identity, bias=OB[:, mg])
                    aOin = ardA.tile([C, L], bf16, tag="aOin")
                    aOout = ardA.tile([C, L], bf16, tag="aOout")
                    nc.sync.dma_start(aOin[:].rearrange("(a p) l -> p a l", p=P), Osb[:])
                    if use_ar:
                        nc.gpsimd.collective_compute("AllReduce", OP.add, replica_groups=group_all,
                                                     ins=[aOin.opt()], outs=[aOout.opt()])
                        nc.sync.dma_start(Osb[:], aOout[:].rearrange("(a p) l -> p a l", p=P))
                    with tc.tile_pool(name="trps", bufs=4, space="PSUM") as tps:
                        for q in range(4):
                            for mg in range(2):
                                for cg in range(2):
                                    tp = tps.tile([P, P], bf16, tag="trp")
                                    src = Osb[:, mg].rearrange("p (a b) -> p a b", b=4)[:, :, q]
                                    nc.tensor.transpose(tp[:], src[:, cg * P:(cg + 1) * P], identb[:])
                                    nc.vector.tensor_copy(hsT[:, cg, q * 256 + mg * P: q * 256 + (mg + 1) * P], tp[:])

                # ===== phase 4: xz projection
                for dt4 in range(4):
                    nc.gpsimd.memset(xh[:, dt4, 0:DC - 1], 0.0)
                with tc.tile_pool(name="p4", bufs=1) as p4:
                    INWT = load(p4, 'INWT', A('INWT').rearrange("(a p) m -> p a m", p=P))
                    with tc.tile_pool(name="xzps", bufs=4, space="PSUM") as xps:
                        for mg in range(8):
                            pt = xps.tile([P, L], f32, tag="xzp")
                            for nh2 in range(2):
                                for kt in range(2):
                                    nc.tensor.matmul(pt[:, NH2[nh2]], (INWT[:, kt, mg * P:(mg + 1) * P]),
                                                     (hsT[:, kt, NH2[nh2]]), start=(kt == 0), stop=(kt == 1))
                            if mg < 4:
                                nc.vector.tensor_copy(xh[:, mg, DC - 1:], pt[:])
                            else:
                                nc.scalar.activation(SZ[:, mg - 4], pt[:], FT.Silu)

            # ===== phase 5: mamba branches (v2; pA closed -> xcn/hsT freed)
            if True:
                with tc.tile_pool(name="brt", bufs=1) as bp, \
                     tc.tile_pool(name="brtmp", bufs=1) as btmp:
                    xms, xdbls, deltas, dus = {}, {}, {}, {}
                    for br in range(3):
                        xms[br] = bp.tile([P, 4, L], bf16, tag=f"xm{br}", name=f"xm{br}")
                        xdbls[br] = bp.tile([48, L], bf16, tag=f"xdbl{br}", name=f"xdbl{br}")
                        deltas[br] = bp.tile([P, 4, L], bf16, tag=f"delta{br}", name=f"delta{br}")
                        dus[br] = bp.tile([P, 4, L], bf16, tag=f"du{br}", name=f"du{br}")
                    # --- 5a: conv1d + silu for all branches
                    with tc.tile_pool(name="xpadp", bufs=1) as xpp, \
                         tc.tile_pool(name="brps", bufs=4, space="PSUM") as bps:
                        for br in range(3):
                            xm = xms[br]
                            if br == 0:
                                xpadv = xh
                            else:
                                xpadv = xpp.tile([P, 4, L + DC - 1], bf16, tag="xpad")
                                for dt4 in range(4):
                                    nc.gpsimd.memset(xpadv[:, dt4, 0:DC - 1], 0.0)
                                    if br == 1:
                                        nc.vector.tensor_copy(xpadv[:, dt4, DC - 1:], xh[:, dt4, DC - 1:][:, ::-1])
                                    else:
                                        nc.vector.tensor_copy(v_jk(xpadv[:, dt4, DC - 1:]), sliced(xh[:, dt4, DC - 1:]))
                            dg = btmp.tile([P, DC, P], bf16, tag="cdiag")
                            for dt4 in range(4):
                                for j in range(DC):
                                    nc.vector.tensor_scalar_mul(dg[:, j], identb[:], CWT[:, dt4, br * DC + j:br * DC + j + 1])
                                pt = bps.tile([P, L], f32, tag="cvp")
                                for nh2 in range(2):
                                    for j in range(DC):
                                        nc.tensor.matmul(pt[:, NH2[nh2]], (dg[:, j]),
                                                         (xpadv[:, dt4, j + nh2 * 512: j + nh2 * 512 + 512]),
                                                         start=(j == 0), stop=(j == DC - 1))
                                nc.scalar.activation(xm[:, dt4], pt[:], FT.Silu,
                                                     bias=CBt[:, dt4, br:br + 1])
                    # --- 5b: x_dbl + softplus + du for all branches
                    with tc.tile_pool(name="xdpp", bufs=2, space="PSUM") as xdpp, \
                         tc.tile_pool(name="dtpp", bufs=2, space="PSUM") as dtpp:
                        for br in range(3):
                            xm, xdbl, delta, du = xms[br], xdbls[br], deltas[br], dus[br]
                            xdp = xdpp.tile([48, L], f32, tag="xdp")
                            for nh2 in range(2):
                                for kt in range(4):
                                    nc.tensor.matmul(xdp[:, NH2[nh2]], (XPWT[:, br, kt]),
                                                     (xm[:, kt, NH2[nh2]]), start=(kt == 0), stop=(kt == 3))
                            nc.vector.tensor_copy(xdbl[:], xdp[:])
                            for dt4 in range(4):
                                dtp = dtpp.tile([P, L], f32, tag="dtp")
                                for nh2 in range(2):
                                    nc.tensor.matmul(dtp[:, NH2[nh2]], (DTWT[:, br, dt4 * P:(dt4 + 1) * P]),
                                                     (xdbl[:DTR, NH2[nh2]]), start=True, stop=True)
                                spe = btmp.tile([P, L], bf16, tag="yf")
                                nc.scalar.activation(spe[:], dtp[:], FT.Exp,
                                                     bias=DTB[:, dt4, br:br + 1])
                                nc.scalar.activation(delta[:, dt4], spe[:], FT.Ln, bias=1.0)
                            nc.vector.tensor_tensor(
                                du[:].rearrange("p a b -> p (a b)"),
                                delta[:].rearrange("p a b -> p (a b)"),
                                xm[:].rearrange("p a b -> p (a b)"), OP.mult)
                    # --- 5c v3: JIT PSUM broadcasts + 4-state chained scans.
                    # The scan chains 4 states in one instruction; zeroing the
                    # first dA column of every state segment makes the chaining
                    # exact (s_0 = dBu_0 regardless of carried state).
                    with tc.tile_pool(name="bcsb", bufs=2) as bcp, \
                         tc.tile_pool(name="sctA", bufs=2) as scpA, \
                         tc.tile_pool(name="sctB", bufs=2) as scpB, \
                         tc.tile_pool(name="sctS", bufs=2) as scpS, \
                         tc.tile_pool(name="bcps", bufs=2, space="PSUM") as bcps, \
                         tc.tile_pool(name="ysps", bufs=1, space="PSUM") as ysps:
                        # pre-zero column 0 of both dA4 buffers once; the exp
                        # activations write only [:, 1:], so the zeros persist
                        # and make the 4-state scan chaining exact.
                        for _ in range(2):
                            t0_ = scpA.tile([P, 4, L], bf16, tag="dA4")
                            nc.vector.memset(t0_[:, :, 0:1], 0.0)
                        for br in range(3):
                            xm, xdbl, delta, du = xms[br], xdbls[br], deltas[br], dus[br]
                            # prebuild B/C broadcast tiles per state-half (shared
                            # across the dt4 loop; bufs=2 pipelines half1 builds
                            # under half0 scans)
                            bch = {}
                            for half in range(nspl // 4):
                                j0 = half * 4
                                bc4 = bcp.tile([P, 4, 2, L], bf16, tag="bc4")
                                bch[half] = bc4
                                for jj in range(4):
                                    j = j0 + jj
                                    brp = bcps.tile([P, L], f32, tag="bcj")
                                    for nh2 in range(2):
                                        nc.tensor.matmul(brp[:, NH2[nh2]], (SELJ[:, j * P:(j + 1) * P]),
                                                         (xdbl[:, NH2[nh2]]), start=True, stop=True)
                                    # br0's copies run on the then-idle vector
                                    # engine; scalar is the prep bottleneck there
                                    if br == 0:
                                        nc.vector.tensor_copy(bc4[:, jj, 0], brp[:])
                                    else:
                                        nc.scalar.copy(bc4[:, jj, 0], brp[:])
                                    crp = bcps.tile([P, L], f32, tag="bcj")
                                    for nh2 in range(2):
                                        nc.tensor.matmul(crp[:, NH2[nh2]], (SELJ[:, (nspl + j) * P:(nspl + j + 1) * P]),
                                                         (xdbl[:, NH2[nh2]]), start=True, stop=True)
                                    if br == 0:
                                        nc.vector.tensor_copy(bc4[:, jj, 1], crp[:])
                                    else:
                                        nc.scalar.copy(bc4[:, jj, 1], crp[:])
                            for dt4 in range(4):
                                yaccp = ysps.tile([P, L], f32, tag="yacc")
                                for half in range(nspl // 4):
                                    j0 = half * 4
                                    bc4 = bch[half]
                                    dA4 = scpA.tile([P, 4, L], bf16, tag="dA4")
                                    for jj in range(4):
                                        j = j0 + jj
                                        nc.scalar.activation(dA4[:, jj, 1:], delta[:, dt4, 1:], FT.Exp,
                                                             scale=ASC[:, dt4, br * nspl + j:br * nspl + j + 1])
                                    dBu4 = scpB.tile([P, 4, L], bf16, tag="dBu4")
                                    for jj in range(4):
                                        nc.vector.tensor_tensor(dBu4[:, jj], du[:, dt4], bc4[:, jj, 0], OP.mult)
                                    sout = scpS.tile([P, 4, L], bf16, tag="sout")
                                    nc.vector.tensor_tensor_scan(
                                        sout[:].rearrange("p a b -> p (a b)"),
                                        dA4[:].rearrange("p a b -> p (a b)"),
                                        dBu4[:].rearrange("p a b -> p (a b)"),
                                        0.0, OP.mult, OP.add)
                                    # ypk = sout * C on gpsimd, written over the consumed dBu4
                                    nc.gpsimd.tensor_tensor(
                                        dBu4[:], sout[:], bc4[:, :, 1], OP.mult)
                                    for jj in range(4):
                                        j = j0 + jj
                                        for nh2 in range(2):
                                            nc.tensor.matmul(yaccp[:, NH2[nh2]], (identb[:]),
                                                             (dBu4[:, jj, NH2[nh2]]),
                                                             start=(j == 0), stop=(j == nspl - 1))
                                # --- 5d for this (br, dt4)
                                yf = btmp.tile([P, L], bf16, tag="yf")
                                nc.vector.scalar_tensor_tensor(yf[:], xm[:, dt4], DPP[:, dt4, br:br + 1],
                                                               yaccp[:], OP.mult, OP.add)
                                if br == 0:
                                    nc.vector.tensor_tensor(comb[:, dt4], yf[:], SZ[:, dt4], OP.mult)
                                elif br == 1:
                                    yg = btmp.tile([P, L], bf16, tag="yg")
                                    nc.gpsimd.tensor_tensor(yg[:], yf[:], SZ[:, dt4][:, ::-1], OP.mult)
                                    nc.vector.tensor_tensor(comb[:, dt4], comb[:, dt4], yg[:][:, ::-1], OP.add)
                                else:
                                    yg = btmp.tile([P, L], bf16, tag="yg")
                                    nc.gpsimd.tensor_tensor(v_jk(yg[:]), v_jk(yf[:]), sliced(SZ[:, dt4]), OP.mult)
                                    nc.vector.tensor_tensor(comb[:, dt4], comb[:, dt4], unsliced(yg[:]), OP.add)

            p5w_cm.__exit__(None, None, None)
            # ===== phases 6+7 share a tail pool (p7 weights prefetch first)
            with tc.tile_pool(name="p7", bufs=1) as p7:
                Mfull = p7.tile([P, 2, L], bf16, tag="Mfull")
                P2T = load(p7, 'P2T', A('P2T').rearrange("(a p) m -> p a m", p=P))
                P2B = load(p7, 'P2B', A('P2B').rearrange("(a p) o -> p a o", p=P))
                F1T = load(p7, 'F1T', A('F1T').rearrange("(a p) m -> p a m", p=P))
                F1B = load(p7, 'F1B')
                DWC = load(p7, 'DWC')
                DWB = load(p7, 'DWB')
                XSKIP = load(p7, 'XSKIP')
                # ===== phase 6: out projection + AllReduce
                with tc.tile_pool(name="p6", bufs=1) as p6, \
                     tc.tile_pool(name="mps", bufs=2, space="PSUM") as mps, \
                     tc.tile_pool(name="ardram", bufs=1, space="DRAM") as ard:
                    OUTWT = load(p6, 'OUTWT', A('OUTWT').rearrange("(a p) m -> p a m", p=P))
                    Mpart = p6.tile([P, 2, L], bf16, tag="mpart")
                    for mg in range(2):
                        mp = mps.tile([P, L], f32, tag="mp")
                        for nh2 in range(2):
                            for kt in range(4):
                                nc.tensor.matmul(mp[:, NH2[nh2]], (OUTWT[:, kt, mg * P:(mg + 1) * P]),
                                                 (comb[:, kt, NH2[nh2]]), start=(kt == 0), stop=(kt == 3))
                        nc.scalar.copy(Mpart[:, mg], mp[:])
                    bin_ = ard.tile([C, L], bf16, tag="arin")
                    bout = ard.tile([C, L], bf16, tag="arout")
                    nc.sync.dma_start(bin_[:].rearrange("(a p) l -> p a l", p=P), Mpart[:])
                    if use_ar:
                        nc.gpsimd.collective_compute("AllReduce", OP.add, replica_groups=group_all,
                                                     ins=[bin_.opt()], outs=[bout.opt()])
                        nc.sync.dma_start(Mfull[:], bout[:].rearrange("(a p) l -> p a l", p=P))
                    else:
                        nc.sync.dma_start(Mfull[:], bin_[:].rearrange("(a p) l -> p a l", p=P))

                # ===== phase 7: conv1#2, conv2, fc1, dw + residual
                # dwg builds first: they are AR-independent vector work and
                # would otherwise queue behind the AR-gated mpad copies
                dwg = p7.tile([P, 9, P], bf16, tag="dwg")
                for t in range(9):
                    nc.vector.tensor_scalar_mul(dwg[:, t], identb[:], DWC[:, t:t + 1])
                mpad = p7.tile([P, 2, HP], bf16, tag="mpad")
                for mg in range(2):
                    nc.gpsimd.memset(mpad[:, mg], 0.0)
                    dst = mpad[:, mg].rearrange("p (h w) -> p h w", h=H + 2)[:, 1:H + 1, 1:W + 1]
                    nc.vector.tensor_copy(dst, Mfull[:, mg].rearrange("p (h w) -> p h w", h=H))
                c1 = p7.tile([P, 2, L], bf16, tag="c1")
                conv3x3(lambda kt: mpad[:, kt], True,
                        lambda mg, nh2: c1[:, mg, NH2[nh2]])
                c2 = p7.tile([P, 2, L], bf16, tag="c2")
                with tc.tile_pool(name="c2ps", bufs=2, space="PSUM") as cps:
                    for mg in range(2):
                        for nh2 in range(2):
                            pt = cps.tile([P, 512], f32, tag="c2p")
                            for kt in range(2):
                                nc.tensor.matmul(pt[:], (P2T[:, kt, mg * P:(mg + 1) * P]),
                                                 (c1[:, kt, NH2[nh2]]), start=(kt == 0), stop=(kt == 1))
                            nc.scalar.activation(c2[:, mg, NH2[nh2]], pt[:], FT.Relu, bias=P2B[:, mg])
                    xfpad = p7.tile([P, HP], bf16, tag="xfpad")
                    nc.gpsimd.memset(xfpad[:], 0.0)
                    for nh2 in range(2):
                        pt = cps.tile([P, 512], f32, tag="fcp")
                        for kt in range(2):
                            nc.tensor.matmul(pt[:], (F1T[:, kt]), (c2[:, kt, NH2[nh2]]),
                                             start=(kt == 0), stop=(kt == 1))
                        dstv = xfpad[:].rearrange("p (h w) -> p h w", h=H + 2)[:, 1 + 16 * nh2:17 + 16 * nh2, 1:W + 1]
                        nc.scalar.activation(dstv, pt[:].rearrange("p (h w) -> p h w", h=16),
                                             FT.Identity, bias=F1B[:])
                    outsb = p7.tile([P, L], f32, tag="outsb")
                    for nh2 in range(2):
                        pt = cps.tile([P, 512], f32, tag="dwp")
                        h0 = 16 * nh2
                        for t in range(9):
                            dy, dx = t // 3, t % 3
                            win = xfpad[:].rearrange("p (h w) -> p h w", h=H + 2)
                            win = win[:, dy + h0:dy + h0 + 16, dx:dx + W]
                            nc.tensor.matmul(pt[:], (dwg[:, t]), (win), start=(t == 0), stop=(t == 8))
                        dwt = p7.tile([P, 512], f32, tag="dwt")
                        nc.scalar.activation(dwt[:], pt[:], FT.Identity, bias=DWB[:])
                        nc.vector.tensor_tensor(outsb[:, NH2[nh2]], dwt[:],
                                                XSKIP[:, NH2[nh2]], OP.add)
                    nc.sync.dma_start(OUTT.ap(), outsb[:])


_CACHE = {}


def _build():
    if 'nc' in _CACHE:
        return
    from concourse import bacc
    nc = bacc.Bacc(target_bir_lowering=False)
    group = [[0, 1], [2, 3], [4, 5], [6, 7]]
    build(nc, use_ar=True, group_all=group, nspl=8)
    nc.compile()
    _CACHE['nc'] = nc


def kernel(**inputs):
    _build()
    from concourse.bass_utils import run_bass_kernel_spmd
    nc = _CACHE['nc']
    in_maps = [host_prep(inputs, core, nspl=8) for core in range(8)]
    res = run_bass_kernel_spmd(nc, in_maps, core_ids=list(range(8)))
    out = np.zeros((B, C, H * W), np.float32)
    for core in range(8):
        b, s = core // 2, core % 2
        out[b, s * 128:(s + 1) * 128] = res.results[core]['OUT']
    return out.reshape(B, C, H, W)

